# revision 31
# baseline (speedup 1.0000x reference)
"""Channel-attention (XCA-style) Trainium2 kernel, 8-way SPMD — v3.

Shapes (hardcoded): B=4, N=16384, D=256, H=2 heads, c=128.
Sharding: core ci -> batch b=ci//2, token half ci%2 (T=8192 tokens/core).

C-matrix factorization: accumulate token-contraction Grams of the scaled
raw inputs (C_rs, C_ss, C_rr + moment vectors against [1, a_t, c_t]),
then assemble attention logits G and the q/k L2 norms as small [256x256]
weight products, pair-AllReduce only [128,260] f32, and collapse the
whole v/attn@v/Wo path into one matrix Pp applied per token from the
d-major transpose of the scaled kv input.

v3: chunk-level DMA transposes, per-chunk stats tiles + deep buffering
for pipelining, batched DRAM bounces, PE warmup chain across the
collective gap.
"""
import sys, types

sys.path.insert(0, "/opt/trn_rl_repo")

try:
    import antenv
    if "antenv.axon_hooks" not in sys.modules:
        _hooks = types.ModuleType("antenv.axon_hooks")
        _hooks._hook = None
        _hooks.set_axon_ntff_profile_hook = lambda h: setattr(_hooks, "_hook", h)
        _hooks.get_axon_ntff_profile_hook = lambda: _hooks._hook
        sys.modules["antenv.axon_hooks"] = _hooks
        antenv.axon_hooks = _hooks
        from trn_agent_boot.trn_boot import _ntff_profile_via_ctypes
        _hooks.set_axon_ntff_profile_hook(
            _ntff_profile_via_ctypes("/opt/axon/libaxon_pjrt.so"))
except Exception:
    pass

import numpy as np
import ml_dtypes

import concourse.bass as bass
import concourse.bacc as bacc
import concourse.mybir as mybir
import concourse.tile as tile
from concourse.bass_utils import run_bass_kernel_spmd

BF16 = ml_dtypes.bfloat16
F32 = mybir.dt.float32
BF = mybir.dt.bfloat16
AL = mybir.AluOpType
AF = mybir.ActivationFunctionType
AX = mybir.AxisListType

B, N, D, H = 4, 16384, 256, 2
T = N // 2                  # tokens per core
NT = 64                     # token tiles per core (inner j), token = p*64 + j
CHT = 8                     # tiles per chunk
NCH = NT // CHT             # 8 chunks
EPS_LN = 1e-5
EPS_NORM = 1e-12
N_CORES = 8
TCORE = float(T)

# stile column layout (bf16): [pad 0:13 | wcol 13:16 | s' 16:272]
WC0 = 13
SP0 = 16
SW = 272

_nc_cache = {}


def _bcast(ap, rows=128):
    return bass.AP(tensor=ap.tensor, offset=ap.offset,
                   ap=[[0, rows]] + [list(x) for x in ap.ap[1:]])


def _build_nc():
    nc = bacc.Bacc("TRN2", target_bir_lowering=False, debug=False,
                   num_devices=N_CORES)

    def ein(name, shape, dt=F32):
        return nc.dram_tensor(name, list(shape), dt, kind="ExternalInput")

    d_s = ein("x_s", [T, D])            # q source shard (input_S)
    d_r = ein("x_r", [T, D])            # kv source shard (input_R)
    d_wqsT = ein("wqsT", [128, 2, D], BF)
    d_wkrT = ein("wkrT", [128, 2, D], BF)
    d_wqs = ein("wqs", [128, 2, D], BF)
    d_wkr = ein("wkr", [128, 2, D], BF)
    d_wvr = ein("wvr", [128, 2, D], BF)
    d_woT = ein("woT", [128, 2, D], BF)
    d_svbv = ein("svbv", [128, 2, 2], BF)
    d_sqk4 = ein("sqk4", [128, 4])      # [sq_h0 sq_h1 sk_h0 sk_h1]
    d_bqk4 = ein("bqk4", [128, 4])      # [bq2 | bk2]
    d_boc = ein("bo_col", [128, 2])
    d_skbk = ein("skbk", [2, 2, 128], BF)
    d_sqbq = ein("sqbq", [2, 2, 128], BF)
    d_eye = ein("eye", [128, 128], BF)
    d_sel4 = ein("sel4", [4, 512], BF)
    d_m24 = ein("m24", [128, 128], BF)
    d_temp = ein("temp", [1, 2])
    d_out = nc.dram_tensor("out", [T, D], BF, kind="ExternalOutput")

    svw = d_s.rearrange("(p j) d -> p j d", p=128)
    rvw = d_r.rearrange("(p j) d -> p j d", p=128)
    outv = d_out.rearrange("(p j) d -> p j d", p=128)

    with tile.TileContext(nc) as tc:
        import contextlib
        with contextlib.ExitStack() as ctx:
            _body(ctx, tc, nc, svw, rvw, outv, d_wqsT, d_wkrT, d_wqs, d_wkr,
                  d_wvr, d_woT, d_svbv, d_sqk4, d_bqk4, d_boc, d_skbk, d_sqbq,
                  d_eye, d_sel4, d_m24, d_temp)
    nc.finalize()
    return nc


def _body(ctx, tc, nc, svw, rvw, outv, d_wqsT, d_wkrT, d_wqs, d_wkr, d_wvr,
          d_woT, d_svbv, d_sqk4, d_bqk4, d_boc, d_skbk, d_sqbq, d_eye, d_sel4, d_m24,
          d_temp):
    E = ctx.enter_context
    consts = E(tc.tile_pool(name="consts", bufs=1))
    stats = E(tc.tile_pool(name="stats", bufs=1))
    ldp = E(tc.tile_pool(name="ldp", bufs=2))
    cbp = E(tc.tile_pool(name="cbp", bufs=3))
    small = E(tc.tile_pool(name="small", bufs=4))
    pers = E(tc.tile_pool(name="pers", bufs=1))
    post = E(tc.tile_pool(name="post", bufs=1))
    outp = E(tc.tile_pool(name="outp", bufs=2))
    dram = E(tc.tile_pool(name="dram", bufs=1, space="DRAM"))

    # ---------------- constants ----------------
    wqsT = consts.tile([128, 2, D], BF, tag="wqsT")
    wkrT = consts.tile([128, 2, D], BF, tag="wkrT")
    wqs = consts.tile([128, 2, D], BF, tag="wqs")
    wkr = consts.tile([128, 2, D], BF, tag="wkr")
    wvr = consts.tile([128, 2, D], BF, tag="wvr")
    woT = consts.tile([128, 2, D], BF, tag="woT")
    for dst, src in ((wqsT, d_wqsT), (wkrT, d_wkrT), (wqs, d_wqs),
                     (wkr, d_wkr), (wvr, d_wvr), (woT, d_woT)):
        nc.sync.dma_start(out=dst[:], in_=src[:, :, :])
    svbv = consts.tile([128, 2, 2], BF, tag="svbv")
    nc.sync.dma_start(out=svbv[:], in_=d_svbv[:, :, :])
    sqk4 = consts.tile([128, 4], F32, tag="sqk4")
    bqk4 = consts.tile([128, 4], F32, tag="bqk4")
    bo_col = consts.tile([128, 2], F32, tag="boc")
    for dst, src in ((sqk4, d_sqk4), (bqk4, d_bqk4), (bo_col, d_boc)):
        nc.sync.dma_start(out=dst[:], in_=src[:, :])
    skbk_rows = consts.tile([2, 2, 128], BF, tag="skbk")
    sqbq_rows = consts.tile([2, 2, 128], BF, tag="sqbq")
    nc.sync.dma_start(out=skbk_rows[:], in_=d_skbk[:, :, :])
    nc.sync.dma_start(out=sqbq_rows[:], in_=d_sqbq[:, :, :])
    eye_sb = consts.tile([128, 128], BF, tag="eye")
    nc.sync.dma_start(out=eye_sb[:], in_=d_eye[:, :])
    sel4 = consts.tile([4, 512], BF, tag="sel4")
    nc.sync.dma_start(out=sel4[:], in_=d_sel4[:, :])
    m24 = consts.tile([128, 128], BF, tag="m24")
    nc.sync.dma_start(out=m24[:], in_=d_m24[:, :])
    ones4 = consts.tile([4, 128], BF, tag="ones4")
    nc.vector.memset(ones4[:], 1.0)
    fm0 = consts.tile([4, 128], BF, tag="fm0")
    fm1 = consts.tile([4, 128], BF, tag="fm1")
    nc.vector.tensor_scalar(fm0[:], sel4[:, 128:256], 1.0, None, AL.mult)
    nc.vector.tensor_scalar(fm1[:], sel4[:, 384:512], 1.0, None, AL.mult)
    temp_b = consts.tile([128, 2], F32, tag="tempb")
    nc.sync.dma_start(out=temp_b[:], in_=_bcast(d_temp[:, :]))
    ones_row = consts.tile([1, 128], BF, tag="ones")
    nc.vector.memset(ones_row[:], 1.0)
    epsln = consts.tile([128, 1], F32, tag="epsln")
    nc.vector.memset(epsln[:], EPS_LN)
    zb = consts.tile([128, 1], F32, tag="zb")
    nc.vector.memset(zb[:], 0.0)

    c_col = stats.tile([128, NT], F32, tag="c_col")
    sqscr = stats.tile([128, 256], BF, tag="sqscr")   # ACT square scratch
    sqscr2 = stats.tile([128, 256], BF, tag="sqscr2")  # DVE square scratch
    rT_all = pers.tile([128, NT, 2, 128], BF, tag="rT")

    # ================= phase A: stream chunks =================
    with tc.tile_pool(name="accA", bufs=1, space="PSUM") as accA:
        b_rs0 = accA.tile([128, 259], F32, tag="b_rs0")
        b_rs1 = accA.tile([128, 259], F32, tag="b_rs1")
        b_ss0 = accA.tile([128, 259], F32, tag="b_ss0")
        b_ss1 = accA.tile([128, 259], F32, tag="b_ss1")
        b_rr0 = accA.tile([128, 256], F32, tag="b_rr0")
        b_rr1 = accA.tile([128, 256], F32, tag="b_rr1")
        b_wg = accA.tile([128, 3], F32, tag="b_wg")

        chunk_state = {}

        def emit_load(ch):
            j0 = ch * CHT
            s_raw = ldp.tile([128, CHT, D], BF, tag="s_raw")
            r_raw = ldp.tile([128, CHT, D], BF, tag="r_raw")
            nc.gpsimd.dma_start(out=s_raw[:], in_=svw[:, j0:j0 + CHT, :])
            nc.gpsimd.dma_start(out=r_raw[:], in_=rvw[:, j0:j0 + CHT, :])
            chunk_state[ch] = {"s_raw": s_raw, "r_raw": r_raw}

        def emit_stats(ch):
            j0 = ch * CHT
            st = chunk_state[ch]
            s_raw, r_raw = st["s_raw"], st["r_raw"]
            stile = cbp.tile([128, CHT, SW], BF, tag="stile")
            rtile = cbp.tile([128, CHT * 256], BF, tag="rtile")
            st["stile"], st["rtile"] = stile, rtile
            bns = small.tile([128, CHT, 6], F32, tag="bns")
            ags = small.tile([128, CHT, 2], F32, tag="ags")
            bnr = small.tile([128, CHT, 6], F32, tag="bnr")
            agr = small.tile([128, CHT, 2], F32, tag="agr")
            for jj in range(CHT):
                nc.vector.bn_stats(bns[:, jj, :], s_raw[:, jj, :])
                nc.vector.bn_stats(bnr[:, jj, :], r_raw[:, jj, :])
            for jj in range(CHT):
                nc.vector.bn_aggr(ags[:, jj, :], bns[:, jj, :])
                nc.vector.bn_aggr(agr[:, jj, :], bnr[:, jj, :])
            sig_s = small.tile([128, CHT], F32, tag="sig_s")
            nc.scalar.activation(out=sig_s[:], in_=ags[:, :, 1], func=AF.Sqrt,
                                 bias=epsln[:, :], scale=1.0)
            sig_r = small.tile([128, CHT], F32, tag="sig_r")
            nc.scalar.activation(out=sig_r[:], in_=agr[:, :, 1], func=AF.Sqrt,
                                 bias=epsln[:, :], scale=1.0)
            st["sig_s"], st["sig_r"] = sig_s, sig_r
            st["mu_s"], st["agr"] = ags, agr
            nc.gpsimd.memset(stile[:, :, WC0], 1.0)

        def emit_compute(ch):
            j0 = ch * CHT
            st = chunk_state.pop(ch)
            s_raw, r_raw = st["s_raw"], st["r_raw"]
            stile, rtile = st["stile"], st["rtile"]
            invs_s = small.tile([128, CHT], F32, tag="invs_s")
            invs_r = small.tile([128, CHT], F32, tag="invs_r")
            nc.vector.reciprocal(out=invs_s[:], in_=st["sig_s"][:])
            nc.vector.reciprocal(out=invs_r[:], in_=st["sig_r"][:])
            nc.vector.scalar_tensor_tensor(
                out=stile[:, :, WC0 + 1], in0=st["mu_s"][:, :, 0],
                scalar=-1.0, op0=AL.mult, op1=AL.mult, in1=invs_s[:])
            nc.vector.scalar_tensor_tensor(
                out=c_col[:, j0:j0 + CHT], in0=st["agr"][:, :, 0], scalar=-1.0,
                op0=AL.mult, op1=AL.mult, in1=invs_r[:])
            nc.scalar.activation(out=stile[:, :, WC0 + 2],
                                 in_=c_col[:, j0:j0 + CHT], func=AF.Copy)
            for jj in range(CHT):
                nc.scalar.activation(
                    out=stile[:, jj, SP0:SP0 + 256], in_=s_raw[:, jj, :],
                    func=AF.Copy, bias=0.0, scale=invs_s[:, jj:jj + 1])
                nc.scalar.activation(
                    out=rtile[:, jj * 256:(jj + 1) * 256],
                    in_=r_raw[:, jj, :], func=AF.Copy, bias=0.0,
                    scale=invs_r[:, jj:jj + 1])
            for jj in range(CHT):
                j = j0 + jj
                fst = (j == 0)
                lst = (j == NT - 1)
                rhs_ws = stile[:, jj, WC0:SP0 + 256]     # [wcol | s'] 259
                rhs_r = rtile[:, jj * 256:(jj + 1) * 256]
                for h in range(2):
                    nc.tensor.matmul(
                        out=(b_rs0 if h == 0 else b_rs1)[:],
                        lhsT=rtile[:, jj * 256 + h * 128:jj * 256 + (h + 1) * 128],
                        rhs=rhs_ws, start=fst, stop=lst)
                for h in range(2):
                    nc.tensor.matmul(
                        out=(b_ss0 if h == 0 else b_ss1)[:],
                        lhsT=stile[:, jj, SP0 + h * 128:SP0 + (h + 1) * 128],
                        rhs=rhs_ws, start=fst, stop=lst)
                for h in range(2):
                    nc.tensor.matmul(
                        out=(b_rr0 if h == 0 else b_rr1)[:],
                        lhsT=rtile[:, jj * 256 + h * 128:jj * 256 + (h + 1) * 128],
                        rhs=rhs_r, start=fst, stop=lst)
                nc.tensor.matmul(out=b_wg[0:3, :],
                                 lhsT=stile[:, jj, WC0:WC0 + 3],
                                 rhs=stile[:, jj, WC0:WC0 + 3],
                                 start=fst, stop=lst)
            nc.sync.dma_start_transpose(rT_all[:, j0:j0 + CHT, :, :],
                                        rtile[:])

        emit_load(0)
        emit_load(1)
        emit_stats(0)
        for ch in range(NCH):
            if ch + 2 < NCH:
                emit_load(ch + 2)
            if ch + 1 < NCH:
                emit_stats(ch + 1)
            emit_compute(ch)

        # ---- evac C matrices (bf16) + S-gram ----
        crs_sb = post.tile([128, 2, 259], BF, tag="crs")
        css_sb = post.tile([128, 2, 259], BF, tag="css")
        crr_sb = post.tile([128, 2, 256], BF, tag="crr")
        sg4_sb = post.tile([4, 3], BF, tag="sg4")
        nc.vector.tensor_scalar(crs_sb[:, 0, :], b_rs0[:], 1.0, None, AL.mult)
        nc.vector.tensor_scalar(crs_sb[:, 1, :], b_rs1[:], 1.0, None, AL.mult)
        nc.scalar.activation(out=css_sb[:, 0, :], in_=b_ss0[:], func=AF.Copy)
        nc.scalar.activation(out=css_sb[:, 1, :], in_=b_ss1[:], func=AF.Copy)
        nc.vector.tensor_scalar(crr_sb[:, 0, :], b_rr0[:], 1.0, None, AL.mult)
        nc.scalar.activation(out=crr_sb[:, 1, :], in_=b_rr1[:], func=AF.Copy)
        nc.vector.memset(sg4_sb[:], 0.0)
        nc.vector.tensor_scalar(sg4_sb[0:3, :], b_wg[0:3, :], 1.0, None,
                                AL.mult)
    # col indices in sgb: Sa=1, Sc=2, Saa=4, Sac=5, Scc=8

    with tc.tile_pool(name="pb", bufs=1, space="PSUM") as pb:
        th4 = pb.tile([128, 4, 3], F32, tag="th4")  # q:[beta|eps|alpha] k:[delta|gam|zeta]
        xh_ps = pb.tile([128, 2, 256], F32, tag="xh")
        g_ps = pb.tile([128, 2, 128], F32, tag="g")
        z_ps = pb.tile([128, 2, 256], F32, tag="z")
        tr_ps = pb.tile([128, 2, 128], BF, tag="tr")
        sgb_ps = pb.tile([128, 3, 3], F32, tag="sgb_ps")
        # S-gram values broadcast to all partitions: 3 indicator matmuls
        for rr2 in range(3):
            nc.tensor.matmul(out=sgb_ps[:, rr2, :],
                             lhsT=sel4[:, rr2 * 128:(rr2 + 1) * 128],
                             rhs=sg4_sb[:, :], start=True, stop=True)
        sgb = post.tile([128, 9], F32, tag="sgb")
        nc.vector.tensor_scalar(sgb[:], sgb_ps[:], 1.0, None, AL.mult)

        for ih in range(2):
            for lh in range(2):
                nc.tensor.matmul(out=th4[:, ih, :],
                                 lhsT=wqsT[:, lh, ih * 128:(ih + 1) * 128],
                                 rhs=css_sb[:, lh, 0:3],
                                 start=(lh == 0), stop=(lh == 1))
                nc.tensor.matmul(out=th4[:, 2 + ih, :],
                                 lhsT=wkrT[:, lh, ih * 128:(ih + 1) * 128],
                                 rhs=crs_sb[:, lh, 0:3],
                                 start=(lh == 0), stop=(lh == 1))
        # Xk_h = Wk_h C_rs   [o in h, j_s(256)]
        for h in range(2):
            for lh in range(2):
                nc.tensor.matmul(out=xh_ps[:, h, :],
                                 lhsT=wkrT[:, lh, h * 128:(h + 1) * 128],
                                 rhs=crs_sb[:, lh, 3:259],
                                 start=(lh == 0), stop=(lh == 1))
        x_sb = post.tile([128, 2, 256], BF, tag="x_sb")
        nc.vector.tensor_scalar(x_sb[:, 0, :], xh_ps[:, 0, :], 1.0, None,
                                AL.mult)
        nc.scalar.activation(out=x_sb[:, 1, :], in_=xh_ps[:, 1, :],
                             func=AF.Copy)
        xT_sb = post.tile([128, 2, 2, 128], BF, tag="xT")
        for h in range(2):
            for jh in range(2):
                nc.tensor.transpose(tr_ps[:, jh, :],
                                    x_sb[:, h, jh * 128:(jh + 1) * 128],
                                    eye_sb[:])
            nc.vector.tensor_scalar(xT_sb[:, h, 0, :], tr_ps[:, 0, :], 1.0,
                                    None, AL.mult)
            nc.scalar.activation(out=xT_sb[:, h, 1, :], in_=tr_ps[:, 1, :],
                                 func=AF.Copy)
        # G_h[i,o] = sum_js WqS[i,js] XkT[js,o]   (rank-1 terms added to
        # the same accumulation group later via K=2 matmuls)
        for h in range(2):
            for jh in range(2):
                nc.tensor.matmul(out=g_ps[:, h, :],
                                 lhsT=wqsT[:, jh, h * 128:(h + 1) * 128],
                                 rhs=xT_sb[:, h, jh, :],
                                 start=(jh == 0), stop=False)
        # d4 = [dq | dk] diag terms
        d4 = small.tile([128, 4], F32, tag="d4")
        dscr = post.tile([128, 256], F32, tag="dscr")
        for a in range(2):
            for lh in range(2):
                nc.tensor.matmul(out=z_ps[:, a, :],
                                 lhsT=wqsT[:, lh, a * 128:(a + 1) * 128],
                                 rhs=css_sb[:, lh, 3:259],
                                 start=(lh == 0), stop=(lh == 1))
        for a in range(2):
            nc.vector.scalar_tensor_tensor(
                out=dscr[:], in0=z_ps[:, a, :], scalar=0.0, op0=AL.bypass,
                op1=AL.mult, in1=wqs[:, a, :], accum_out=d4[:, a:a + 1])
        for a in range(2):
            for lh in range(2):
                nc.tensor.matmul(out=z_ps[:, a, :],
                                 lhsT=wkrT[:, lh, a * 128:(a + 1) * 128],
                                 rhs=crr_sb[:, lh, :],
                                 start=(lh == 0), stop=(lh == 1))
        for a in range(2):
            nc.vector.scalar_tensor_tensor(
                out=dscr[:], in0=z_ps[:, a, :], scalar=0.0, op0=AL.bypass,
                op1=AL.mult, in1=wkr[:, a, :], accum_out=d4[:, 2 + a:3 + a])

        # norms (q and k combined on [128,4]):
        # n = d + 2*th[...,1]*u + 2*th[...,0]*g + sXX*u^2 + 2*sX*u*g + T*g^2
        nqk2 = small.tile([128, 4], F32, tag="nqk2")
        t1 = small.tile([128, 4], F32, tag="t1")
        t2 = small.tile([128, 4], F32, tag="t2")
        sXX4 = small.tile([128, 4], F32, tag="sXX4")
        sX4 = small.tile([128, 4], F32, tag="sX4")
        for cdst, csrc in ((sXX4[:, 0:2], 4), (sXX4[:, 2:4], 8),
                           (sX4[:, 0:2], 1), (sX4[:, 2:4], 2)):
            nc.vector.tensor_scalar(cdst, _bcfree(sgb, csrc, 2), 1.0, None,
                                    AL.mult)
        nc.vector.tensor_tensor(out=t1[:, 0:2], in0=th4[:, 0:2, 1],
                                in1=sqk4[:, 0:2], op=AL.mult)
        nc.vector.tensor_tensor(out=t1[:, 2:4], in0=th4[:, 2:4, 2],
                                in1=sqk4[:, 2:4], op=AL.mult)
        nc.vector.scalar_tensor_tensor(out=nqk2[:], in0=t1[:], scalar=2.0,
                                       op0=AL.mult, op1=AL.add, in1=d4[:])
        nc.vector.tensor_tensor(out=t1[:], in0=th4[:, :, 0], in1=bqk4[:],
                                op=AL.mult)
        nc.vector.scalar_tensor_tensor(out=nqk2[:], in0=t1[:], scalar=2.0,
                                       op0=AL.mult, op1=AL.add, in1=nqk2[:])
        nc.vector.tensor_tensor(out=t1[:], in0=sqk4[:], in1=sqk4[:],
                                op=AL.mult)
        nc.vector.tensor_tensor(out=t2[:], in0=t1[:], in1=sXX4[:], op=AL.mult)
        nc.vector.tensor_tensor(out=nqk2[:], in0=nqk2[:], in1=t2[:], op=AL.add)
        nc.vector.tensor_tensor(out=t1[:], in0=sqk4[:], in1=bqk4[:],
                                op=AL.mult)
        nc.vector.tensor_tensor(out=t2[:], in0=t1[:], in1=sX4[:], op=AL.mult)
        nc.vector.scalar_tensor_tensor(out=nqk2[:], in0=t2[:], scalar=2.0,
                                       op0=AL.mult, op1=AL.add, in1=nqk2[:])
        nc.vector.tensor_tensor(out=t1[:], in0=bqk4[:], in1=bqk4[:],
                                op=AL.mult)
        nc.vector.scalar_tensor_tensor(out=nqk2[:], in0=t1[:], scalar=TCORE,
                                       op0=AL.mult, op1=AL.add, in1=nqk2[:])

        # G rank-1 rows (k-side combos), broadcast fully on-chip
        r12c = small.tile([128, 2, 2], BF, tag="r12c")  # [m(row1/2), h]
        nc.vector.scalar_tensor_tensor(out=r12c[:, 0, :], in0=sqk4[:, 2:4],
                                       scalar=sgb[:, 5:6], op0=AL.mult,
                                       op1=AL.add, in1=th4[:, 2:4, 1])
        nc.vector.scalar_tensor_tensor(out=r12c[:, 0, :], in0=bqk4[:, 2:4],
                                       scalar=sgb[:, 1:2], op0=AL.mult,
                                       op1=AL.add, in1=r12c[:, 0, :])
        nc.vector.scalar_tensor_tensor(out=r12c[:, 1, :], in0=sqk4[:, 2:4],
                                       scalar=sgb[:, 2:3], op0=AL.mult,
                                       op1=AL.add, in1=th4[:, 2:4, 0])
        nc.vector.scalar_tensor_tensor(out=r12c[:, 1, :], in0=bqk4[:, 2:4],
                                       scalar=TCORE, op0=AL.mult,
                                       op1=AL.add, in1=r12c[:, 1, :])
        # per head: pack cols (alpha, beta | r1, r2), transpose to rows,
        # then two K=2 rank-1 matmuls finish the G accumulation group
        abr_sb = post.tile([128, 2, 4], BF, tag="abr")
        for h in range(2):
            nc.vector.tensor_scalar(abr_sb[:, h, 0:1], th4[:, h, 2:3], 1.0,
                                    None, AL.mult)
            nc.vector.tensor_scalar(abr_sb[:, h, 1:2], th4[:, h, 0:1], 1.0,
                                    None, AL.mult)
        nc.vector.tensor_scalar(abr_sb[:, :, 2], r12c[:, 0, :], 1.0, None,
                                AL.mult)
        nc.vector.tensor_scalar(abr_sb[:, :, 3], r12c[:, 1, :], 1.0, None,
                                AL.mult)
        ab_row = post.tile([2, 2, 128], BF, tag="ab_row")
        r12_row = post.tile([2, 2, 128], BF, tag="r12_row")
        for h in range(2):
            nc.tensor.transpose(tr_ps[0:2, 0, :], abr_sb[:, h, 0:2],
                                eye_sb[:])
            nc.tensor.transpose(tr_ps[0:2, 1, :], abr_sb[:, h, 2:4],
                                eye_sb[:])
            nc.scalar.activation(out=ab_row[:, h, :], in_=tr_ps[0:2, 0, :],
                                 func=AF.Copy)
            nc.scalar.activation(out=r12_row[:, h, :], in_=tr_ps[0:2, 1, :],
                                 func=AF.Copy)
        for h in range(2):
            nc.tensor.matmul(out=g_ps[:, h, :], lhsT=ab_row[:, h, :],
                             rhs=skbk_rows[:, h, :], start=False, stop=False)
            nc.tensor.matmul(out=g_ps[:, h, :], lhsT=sqbq_rows[:, h, :],
                             rhs=r12_row[:, h, :], start=False, stop=True)

        # pack [G0 | G1 | nq2 | nk2]  (bf16 collective)
        pack = post.tile([128, 260], BF, tag="pack")
        nc.scalar.activation(out=pack[:, 0:128], in_=g_ps[:, 0, :],
                             func=AF.Copy)
        nc.scalar.activation(out=pack[:, 128:256], in_=g_ps[:, 1, :],
                             func=AF.Copy)
        nc.vector.tensor_scalar(pack[:, 256:260], nqk2[:], 1.0, None, AL.mult)

    cc_in = dram.tile([128, 260], BF)
    cc_out = dram.tile([128, 260], BF)
    nc.sync.dma_start(out=cc_in[:, :], in_=pack[:])
    nc.gpsimd.collective_compute(
        "AllReduce", AL.add,
        replica_groups=[[0, 1], [2, 3], [4, 5], [6, 7]],
        ins=[cc_in.opt()], outs=[cc_out.opt()])

    red = post.tile([128, 260], BF, tag="red")
    nc.sync.dma_start(out=red[:], in_=cc_out[:, :])

    # ================= phase C: softmax + Pp/f assembly ================
    with tc.tile_pool(name="pc", bufs=1, space="PSUM") as pc2:
        e_ps = pc2.tile([128, 2, 2, 128], F32, tag="e_ps")
        # --- PE warmup chain to keep HAM hot across the collective gap ---
        wu_sb = post.tile([128, 128], BF, tag="wu_sb")
        nc.vector.tensor_scalar(wu_sb[:], eye_sb[:], 1.0, None, AL.mult)

        def warmup(n):
            for k in range(n):
                nc.tensor.matmul(out=e_ps[:, 0, 0, :], lhsT=wu_sb[:],
                                 rhs=eye_sb[:], start=True, stop=True)
                nc.scalar.activation(out=wu_sb[:], in_=e_ps[:, 0, 0, :],
                                     func=AF.Copy)

        warmup(18)

        trx_ps = pc2.tile([128, 6, 128], BF, tag="trx")
        tr2_ps = trx_ps[:, 0:2, :]
        invq = small.tile([128, 2], F32, tag="invq")
        invk = small.tile([128, 2], F32, tag="invk")
        for dst, src_off, mul_temp in ((invq, 256, True), (invk, 258, False)):
            sq_ = small.tile([128, 2], F32, tag="invn_sq")
            nc.scalar.activation(out=sq_[:], in_=red[:, src_off:src_off + 2],
                                 func=AF.Sqrt, bias=zb[:, :], scale=1.0)
            nc.vector.tensor_scalar_max(sq_[:], sq_[:], EPS_NORM)
            nc.vector.reciprocal(out=dst[:], in_=sq_[:])
            if mul_temp:
                nc.vector.tensor_tensor(out=dst[:], in0=dst[:],
                                        in1=temp_b[:, :], op=AL.mult)
        invk_bf = small.tile([128, 2], BF, tag="invk_bf")
        nc.vector.tensor_scalar(invk_bf[:], invk[:], 1.0, None, AL.mult)
        nc.tensor.transpose(tr2_ps[0:2, 0, :], invk_bf[:], eye_sb[:])
        ik2_sb = post.tile([2, 128], BF, tag="ik2")
        nc.scalar.activation(out=ik2_sb[:], in_=tr2_ps[0:2, 0, :],
                             func=AF.Copy)
        iktf_ps = pc2.tile([128, 2, 132], F32, tag="iktf_ps")
        ikb_ps = iktf_ps[:, :, 0:128]
        for h in range(2):
            nc.tensor.matmul(out=ikb_ps[:, h, :],
                             lhsT=sel4[0:2, h * 128:(h + 1) * 128],
                             rhs=ik2_sb[:], start=True, stop=True)

        a_sb = post.tile([128, 2, 128], BF, tag="a_sb")
        esc = post.tile([128, 2, 128], F32, tag="esc")
        for h in range(2):
            lh_t = post.tile([128, 128], F32, tag="lh_t")
            nc.vector.tensor_scalar(lh_t[:], red[:, h * 128:(h + 1) * 128],
                                    invq[:, h:h + 1], None, AL.mult)
            nc.vector.tensor_tensor(out=lh_t[:], in0=lh_t[:],
                                    in1=ikb_ps[:, h, :], op=AL.mult)
            rmax = small.tile([128, 1], F32, tag="rmax")
            nc.vector.tensor_reduce(out=rmax[:], in_=lh_t[:], op=AL.max,
                                    axis=AX.X)
            nc.vector.tensor_scalar(rmax[:], rmax[:], -1.0, None, AL.mult)
            rsum = small.tile([128, 1], F32, tag="rsum")
            nc.scalar.activation(out=esc[:, h, :], in_=lh_t[:], func=AF.Exp,
                                 bias=rmax[:, :], scale=1.0,
                                 accum_out=rsum[:])
            nc.vector.reciprocal(out=rsum[:], in_=rsum[:])
            nc.vector.tensor_scalar(a_sb[:, h, :], esc[:, h, :],
                                    rsum[:, :], None, AL.mult)

        attnT = post.tile([128, 2, 128], BF, tag="attnT")
        for h in range(2):
            nc.tensor.transpose(tr2_ps[:, h, :], a_sb[:, h, :], eye_sb[:])
        for h in range(2):
            nc.scalar.activation(out=attnT[:, h, :], in_=tr2_ps[:, h, :],
                                 func=AF.Copy)

        for h in range(2):
            for ph in range(2):
                nc.tensor.matmul(out=e_ps[:, ph, h, :],
                                 lhsT=woT[:, h, ph * 128:(ph + 1) * 128],
                                 rhs=a_sb[:, h, :], start=True, stop=True)
        e_sb = post.tile([128, 2, 2, 128], BF, tag="e_sb")
        for ph in range(2):
            nc.vector.tensor_scalar(e_sb[:, ph, 0, :], e_ps[:, ph, 0, :],
                                    1.0, None, AL.mult)
            nc.scalar.activation(out=e_sb[:, ph, 1, :], in_=e_ps[:, ph, 1, :],
                                 func=AF.Copy)
        et_ps = trx_ps[:, 2:6, :]
        eT_sb = post.tile([128, 2, 256], BF, tag="eT")
        for h in range(2):
            for ph in range(2):
                nc.tensor.transpose(et_ps[:, h * 2 + ph, :],
                                    e_sb[:, ph, h, :], eye_sb[:])
        for h in range(2):
            nc.vector.tensor_scalar(eT_sb[:, h, 0:128],
                                    et_ps[:, h * 2 + 0, :],
                                    1.0, None, AL.mult)
            nc.scalar.activation(out=eT_sb[:, h, 128:256],
                                 in_=et_ps[:, h * 2 + 1, :], func=AF.Copy)
        ppt_ps = pc2.tile([128, 2, 256], F32, tag="ppt")
        for mh in range(2):
            for h in range(2):
                nc.tensor.matmul(out=ppt_ps[:, mh, :],
                                 lhsT=wvr[:, h, mh * 128:(mh + 1) * 128],
                                 rhs=eT_sb[:, h, :],
                                 start=(h == 0), stop=(h == 1))
        pptT = post.tile([128, 2, 256], BF, tag="pptT")
        nc.vector.tensor_scalar(pptT[:, 0, :], ppt_ps[:, 0, :], 1.0, None,
                                AL.mult)
        nc.scalar.activation(out=pptT[:, 1, :], in_=ppt_ps[:, 1, :],
                             func=AF.Copy)

        t_ps = iktf_ps[:, :, 128:130]
        f12_ps = iktf_ps[:, :, 130:132]
        for h in range(2):
            nc.tensor.matmul(out=t_ps[:, h, :], lhsT=attnT[:, h, :],
                             rhs=svbv[:, h, :], start=True, stop=True)
        t_sb = post.tile([128, 2, 2], BF, tag="t_sb")
        nc.vector.tensor_scalar(t_sb[:], t_ps[:, :, :], 1.0, None, AL.mult)
        for ph in range(2):
            for h in range(2):
                nc.tensor.matmul(out=f12_ps[:, ph, :],
                                 lhsT=woT[:, h, ph * 128:(ph + 1) * 128],
                                 rhs=t_sb[:, h, :],
                                 start=(h == 0), stop=(h == 1))
        f12_sb = post.tile([128, 2, 2], BF, tag="f12sb")
        nc.vector.tensor_scalar(f12_sb[:, :, 0], f12_ps[:, :, 0], 1.0, None,
                                AL.mult)
        nc.vector.tensor_tensor(out=f12_sb[:, :, 1], in0=f12_ps[:, :, 1],
                                in1=bo_col[:, :], op=AL.add)
        # f rows: transpose [128,(ph,m)] -> [4,128] (k=ph*2+m), then
        # indicator-MM broadcasts; f2 becomes a K=4 block rhs for phase D
        nc.tensor.transpose(tr2_ps[0:4, 0, :], f12_sb[:, :, :], eye_sb[:])
        f4_sb = post.tile([4, 128], BF, tag="f4_sb")
        nc.scalar.activation(out=f4_sb[:], in_=tr2_ps[0:4, 0, :], func=AF.Copy)
        for ph in range(2):
            nc.tensor.matmul(out=ikb_ps[:, ph, :],
                             lhsT=sel4[:, (ph * 2) * 128:(ph * 2 + 1) * 128],
                             rhs=f4_sb[:], start=True, stop=True)
        f1b = post.tile([128, 256], F32, tag="f1b")
        nc.vector.tensor_scalar(f1b[:], ikb_ps[:, :, :], 1.0, None, AL.mult)
        f24_sb = post.tile([128, 256], BF, tag="f24")
        nc.vector.memset(f24_sb[:], 0.0)
        nc.vector.tensor_tensor(out=f24_sb[0:4, 0:128], in0=f4_sb[:],
                                in1=fm0[:], op=AL.mult)
        nc.vector.tensor_tensor(out=f24_sb[0:4, 128:256], in0=f4_sb[:],
                                in1=fm1[:], op=AL.mult)

        # ============= phase D: output pass (same psum pool) =============
        opsum0 = pc2.tile([128, 256], F32, tag="opsum0")
        opsum1 = pc2.tile([128, 256], F32, tag="opsum1")
        opsum2 = pc2.tile([128, 256], F32, tag="opsum2")
        opsum3 = pc2.tile([128, 256], F32, tag="opsum3")
        op_t = [opsum0, opsum1, opsum2, opsum3]
        for g in range(NT // 4):
            j0 = g * 4
            out_sb = outp.tile([128, 4, 256], BF, tag="out_sb")
            for jj in range(4):
                j = j0 + jj
                opsum = op_t[j % 4]
                nc.tensor.matmul(out=opsum[:], lhsT=rT_all[:, j, 0, :],
                                 rhs=pptT[:, 0, :], start=True, stop=False)
                nc.tensor.matmul(out=opsum[:], lhsT=rT_all[:, j, 1, :],
                                 rhs=pptT[:, 1, :], start=False, stop=False)
                nc.tensor.matmul(out=opsum[:], lhsT=m24[:, :],
                                 rhs=f24_sb[:, :], start=False, stop=True)
                nc.vector.scalar_tensor_tensor(
                    out=out_sb[:, jj, :], in0=f1b[:],
                    scalar=c_col[:, j:j + 1], op0=AL.mult, op1=AL.add,
                    in1=opsum[:])
            nc.sync.dma_start(out=outv[:, j0:j0 + 4, :], in_=out_sb[:])


def _bcfree(tile_, col, n):
    """AP reading tile_[:, col] broadcast n times along free (0-stride)."""
    ap = tile_[:, col:col + 1]
    return bass.AP(tensor=ap.tensor, offset=ap.offset,
                   ap=[list(ap.ap[0])] + [[0, n]])


# ======================= host side =======================

def _sel4():
    s = np.zeros((4, 512), np.float32)
    for k in range(4):
        s[k, k * 128:(k + 1) * 128] = 1.0
    return s.astype(BF16)


def _m24():
    m = np.zeros((128, 128), np.float32)
    m[1, :] = 1.0
    m[3, :] = 1.0
    return m.astype(BF16)


def _prep_shared(inputs):
    f32 = np.float32
    Wq = np.asarray(inputs["Wq"], f32)
    bq = np.asarray(inputs["bq"], f32)
    Wkv = np.asarray(inputs["Wkv"], f32)
    bkv = np.asarray(inputs["bkv"], f32)
    Wo = np.asarray(inputs["Wo"], f32)
    bo = np.asarray(inputs["bo"], f32)
    lnS_w = np.asarray(inputs["lnS_w"], f32)
    lnS_b = np.asarray(inputs["lnS_b"], f32)
    lnR_w = np.asarray(inputs["lnR_w"], f32)
    lnR_b = np.asarray(inputs["lnR_b"], f32)
    temp = np.asarray(inputs["temperature"], f32).reshape(H)

    Wk, Wv = Wkv[:D], Wkv[D:]
    WqS = Wq * lnS_w[None, :]
    WkR = Wk * lnR_w[None, :]
    WvR = Wv * lnR_w[None, :]
    sq = WqS.sum(1)
    sk = WkR.sum(1)
    sv = WvR.sum(1)
    bq2 = Wq @ lnS_b + bq
    bk2 = Wk @ lnR_b + bkv[:D]
    bv2 = Wv @ lnR_b + bkv[D:]

    def halved(M):  # [256, X] -> [128, 2, X] rows split into halves
        return np.ascontiguousarray(
            M.reshape(2, 128, M.shape[1]).transpose(1, 0, 2)).astype(BF16)

    def colh(v):
        return np.ascontiguousarray(v.reshape(2, 128).T, f32)

    svbv = np.stack([sv, bv2], 1)  # [256, 2]
    sqk4 = np.concatenate([colh(sq), colh(sk)], 1)
    bqk4 = np.concatenate([colh(bq2), colh(bk2)], 1)
    return {
        "wqsT": halved(np.ascontiguousarray(WqS.T)),
        "wkrT": halved(np.ascontiguousarray(WkR.T)),
        "wqs": halved(WqS),
        "wkr": halved(WkR),
        "wvr": halved(WvR),
        "woT": halved(np.ascontiguousarray(Wo.T)),
        "svbv": halved(svbv),
        "sqk4": np.ascontiguousarray(sqk4),
        "bqk4": np.ascontiguousarray(bqk4),
        "bo_col": colh(bo),
        "skbk": np.stack([sk.reshape(2, 128),
                          bk2.reshape(2, 128)], 0).astype(BF16),
        "sqbq": np.stack([sq.reshape(2, 128),
                          bq2.reshape(2, 128)], 0).astype(BF16),
        "eye": np.eye(128).astype(BF16),
        "sel4": _sel4(),
        "m24": _m24(),
        "temp": temp.reshape(1, H).astype(f32),
    }


def _get_nc():
    if "nc" not in _nc_cache:
        _nc_cache["nc"] = _build_nc()
    return _nc_cache["nc"]


def run(inputs, trace=False):
    nc = _get_nc()
    shared = _prep_shared(inputs)
    iR = np.asarray(inputs["input_R"], np.float32)
    iS = np.asarray(inputs["input_S"], np.float32)
    in_maps = []
    for ci in range(N_CORES):
        b, half = ci // 2, ci % 2
        m = dict(shared)
        m["x_r"] = np.ascontiguousarray(iR[b, half * T:(half + 1) * T])
        m["x_s"] = np.ascontiguousarray(iS[b, half * T:(half + 1) * T])
        in_maps.append(m)
    res = run_bass_kernel_spmd(nc, in_maps, list(range(N_CORES)), trace=trace)
    out = np.zeros((B, N, D), np.float32)
    for ci in range(N_CORES):
        b, half = ci // 2, ci % 2
        out[b, half * T:(half + 1) * T] = np.asarray(
            res.results[ci]["out"]).astype(np.float32)
    return out, res


def kernel(**inputs):
    out, _ = run(inputs, trace=False)
    return out


# revision 32
# speedup vs baseline: 1.1489x; 1.1489x over previous
"""Channel-attention (XCA-style) Trainium2 kernel, 8-way SPMD — v3.

Shapes (hardcoded): B=4, N=16384, D=256, H=2 heads, c=128.
Sharding: core ci -> batch b=ci//2, token half ci%2 (T=8192 tokens/core).

C-matrix factorization: accumulate token-contraction Grams of the scaled
raw inputs (C_rs, C_ss, C_rr + moment vectors against [1, a_t, c_t]),
then assemble attention logits G and the q/k L2 norms as small [256x256]
weight products, pair-AllReduce only [128,260] f32, and collapse the
whole v/attn@v/Wo path into one matrix Pp applied per token from the
d-major transpose of the scaled kv input.

v3: chunk-level DMA transposes, per-chunk stats tiles + deep buffering
for pipelining, batched DRAM bounces, PE warmup chain across the
collective gap.
"""
import sys, types

sys.path.insert(0, "/opt/trn_rl_repo")

try:
    import antenv
    if "antenv.axon_hooks" not in sys.modules:
        _hooks = types.ModuleType("antenv.axon_hooks")
        _hooks._hook = None
        _hooks.set_axon_ntff_profile_hook = lambda h: setattr(_hooks, "_hook", h)
        _hooks.get_axon_ntff_profile_hook = lambda: _hooks._hook
        sys.modules["antenv.axon_hooks"] = _hooks
        antenv.axon_hooks = _hooks
        from trn_agent_boot.trn_boot import _ntff_profile_via_ctypes
        _hooks.set_axon_ntff_profile_hook(
            _ntff_profile_via_ctypes("/opt/axon/libaxon_pjrt.so"))
except Exception:
    pass

import numpy as np
import ml_dtypes

import concourse.bass as bass
import concourse.bacc as bacc
import concourse.mybir as mybir
import concourse.tile as tile
from concourse.bass_utils import run_bass_kernel_spmd

BF16 = ml_dtypes.bfloat16
F32 = mybir.dt.float32
BF = mybir.dt.bfloat16
AL = mybir.AluOpType
AF = mybir.ActivationFunctionType
AX = mybir.AxisListType

B, N, D, H = 4, 16384, 256, 2
T = N // 2                  # tokens per core
NT = 64                     # token tiles per core (inner j), token = p*64 + j
CHT = 8                     # tiles per chunk
NCH = NT // CHT             # 8 chunks
EPS_LN = 1e-5
EPS_NORM = 1e-12
N_CORES = 8
TCORE = float(T)

# stile column layout (bf16): [pad 0:13 | wcol 13:16 | s' 16:272]
WC0 = 13
SP0 = 16
SW = 272

_nc_cache = {}


def _bcast(ap, rows=128):
    return bass.AP(tensor=ap.tensor, offset=ap.offset,
                   ap=[[0, rows]] + [list(x) for x in ap.ap[1:]])


def _build_nc():
    nc = bacc.Bacc("TRN2", target_bir_lowering=False, debug=False,
                   num_devices=N_CORES)

    def ein(name, shape, dt=F32):
        return nc.dram_tensor(name, list(shape), dt, kind="ExternalInput")

    d_s = ein("x_s", [T, D])            # q source shard (input_S)
    d_r = ein("x_r", [T, D])            # kv source shard (input_R)
    d_wqsT = ein("wqsT", [128, 2, D], BF)
    d_wkrT = ein("wkrT", [128, 2, D], BF)
    d_wqs = ein("wqs", [128, 2, D], BF)
    d_wkr = ein("wkr", [128, 2, D], BF)
    d_wvr = ein("wvr", [128, 2, D], BF)
    d_woT = ein("woT", [128, 2, D], BF)
    d_svbv = ein("svbv", [128, 2, 2], BF)
    d_sqk4 = ein("sqk4", [128, 4])      # [sq_h0 sq_h1 sk_h0 sk_h1]
    d_bqk4 = ein("bqk4", [128, 4])      # [bq2 | bk2]
    d_boc = ein("bo_col", [128, 2])
    d_skbk = ein("skbk", [2, 2, 128], BF)
    d_sqbq = ein("sqbq", [2, 2, 128], BF)
    d_eye = ein("eye", [128, 128], BF)
    d_sel4 = ein("sel4", [4, 512], BF)
    d_m24 = ein("m24", [128, 128], BF)
    d_temp = ein("temp", [1, 2])
    d_out = nc.dram_tensor("out", [T, D], BF, kind="ExternalOutput")

    svw = d_s.rearrange("(p j) d -> p j d", p=128)
    rvw = d_r.rearrange("(p j) d -> p j d", p=128)
    outv = d_out.rearrange("(p j) d -> p j d", p=128)

    with tile.TileContext(nc) as tc:
        import contextlib
        with contextlib.ExitStack() as ctx:
            _body(ctx, tc, nc, svw, rvw, outv, d_wqsT, d_wkrT, d_wqs, d_wkr,
                  d_wvr, d_woT, d_svbv, d_sqk4, d_bqk4, d_boc, d_skbk, d_sqbq,
                  d_eye, d_sel4, d_m24, d_temp)
    nc.finalize()
    return nc


def _body(ctx, tc, nc, svw, rvw, outv, d_wqsT, d_wkrT, d_wqs, d_wkr, d_wvr,
          d_woT, d_svbv, d_sqk4, d_bqk4, d_boc, d_skbk, d_sqbq, d_eye, d_sel4, d_m24,
          d_temp):
    E = ctx.enter_context
    consts = E(tc.tile_pool(name="consts", bufs=1))
    stats = E(tc.tile_pool(name="stats", bufs=1))
    ldp = E(tc.tile_pool(name="ldp", bufs=3))
    cbp = E(tc.tile_pool(name="cbp", bufs=3))
    small = E(tc.tile_pool(name="small", bufs=4))
    pers = E(tc.tile_pool(name="pers", bufs=1))
    post = E(tc.tile_pool(name="post", bufs=1))
    outp = E(tc.tile_pool(name="outp", bufs=2))
    dram = E(tc.tile_pool(name="dram", bufs=1, space="DRAM"))

    # ---------------- constants ----------------
    wqsT = consts.tile([128, 2, D], BF, tag="wqsT")
    wkrT = consts.tile([128, 2, D], BF, tag="wkrT")
    wqs = consts.tile([128, 2, D], BF, tag="wqs")
    wkr = consts.tile([128, 2, D], BF, tag="wkr")
    wvr = consts.tile([128, 2, D], BF, tag="wvr")
    woT = consts.tile([128, 2, D], BF, tag="woT")
    for dst, src in ((wqsT, d_wqsT), (wkrT, d_wkrT), (wqs, d_wqs),
                     (wkr, d_wkr), (wvr, d_wvr), (woT, d_woT)):
        nc.sync.dma_start(out=dst[:], in_=src[:, :, :])
    svbv = consts.tile([128, 2, 2], BF, tag="svbv")
    nc.sync.dma_start(out=svbv[:], in_=d_svbv[:, :, :])
    sqk4 = consts.tile([128, 4], F32, tag="sqk4")
    bqk4 = consts.tile([128, 4], F32, tag="bqk4")
    bo_col = consts.tile([128, 2], F32, tag="boc")
    for dst, src in ((sqk4, d_sqk4), (bqk4, d_bqk4), (bo_col, d_boc)):
        nc.sync.dma_start(out=dst[:], in_=src[:, :])
    skbk_rows = consts.tile([2, 2, 128], BF, tag="skbk")
    sqbq_rows = consts.tile([2, 2, 128], BF, tag="sqbq")
    nc.sync.dma_start(out=skbk_rows[:], in_=d_skbk[:, :, :])
    nc.sync.dma_start(out=sqbq_rows[:], in_=d_sqbq[:, :, :])
    eye_sb = consts.tile([128, 128], BF, tag="eye")
    nc.sync.dma_start(out=eye_sb[:], in_=d_eye[:, :])
    sel4 = consts.tile([4, 512], BF, tag="sel4")
    nc.sync.dma_start(out=sel4[:], in_=d_sel4[:, :])
    m24 = consts.tile([128, 128], BF, tag="m24")
    nc.sync.dma_start(out=m24[:], in_=d_m24[:, :])
    ones4 = consts.tile([4, 128], BF, tag="ones4")
    nc.vector.memset(ones4[:], 1.0)
    fm0 = consts.tile([4, 128], BF, tag="fm0")
    fm1 = consts.tile([4, 128], BF, tag="fm1")
    nc.vector.tensor_scalar(fm0[:], sel4[:, 128:256], 1.0, None, AL.mult)
    nc.vector.tensor_scalar(fm1[:], sel4[:, 384:512], 1.0, None, AL.mult)
    temp_b = consts.tile([128, 2], F32, tag="tempb")
    nc.sync.dma_start(out=temp_b[:], in_=_bcast(d_temp[:, :]))
    ones_row = consts.tile([1, 128], BF, tag="ones")
    nc.vector.memset(ones_row[:], 1.0)
    epsln = consts.tile([128, 1], F32, tag="epsln")
    nc.vector.memset(epsln[:], EPS_LN)
    zb = consts.tile([128, 1], F32, tag="zb")
    nc.vector.memset(zb[:], 0.0)

    c_col = stats.tile([128, NT], F32, tag="c_col")
    sqscr = stats.tile([128, 256], BF, tag="sqscr")   # ACT square scratch
    sqscr2 = stats.tile([128, 256], BF, tag="sqscr2")  # DVE square scratch
    rT_all = pers.tile([128, NT, 2, 128], BF, tag="rT")

    # ================= phase A: stream chunks =================
    with tc.tile_pool(name="accA", bufs=1, space="PSUM") as accA:
        b_rs0 = accA.tile([128, 259], F32, tag="b_rs0")
        b_rs1 = accA.tile([128, 259], F32, tag="b_rs1")
        b_ss0 = accA.tile([128, 259], F32, tag="b_ss0")
        b_ss1 = accA.tile([128, 259], F32, tag="b_ss1")
        b_rr0 = accA.tile([128, 256], F32, tag="b_rr0")
        b_rr1 = accA.tile([128, 256], F32, tag="b_rr1")
        b_wg = accA.tile([128, 3], F32, tag="b_wg")

        chunk_state = {}

        def emit_load(ch):
            j0 = ch * CHT
            s_raw = ldp.tile([128, CHT, D], BF, tag="s_raw")
            r_raw = ldp.tile([128, CHT, D], BF, tag="r_raw")
            nc.gpsimd.dma_start(out=s_raw[:], in_=svw[:, j0:j0 + CHT, :])
            nc.gpsimd.dma_start(out=r_raw[:], in_=rvw[:, j0:j0 + CHT, :])
            chunk_state[ch] = {"s_raw": s_raw, "r_raw": r_raw}

        def emit_stats(ch):
            j0 = ch * CHT
            st = chunk_state[ch]
            s_raw, r_raw = st["s_raw"], st["r_raw"]
            stile = cbp.tile([128, CHT, SW], BF, tag="stile")
            rtile = cbp.tile([128, CHT * 256], BF, tag="rtile")
            st["stile"], st["rtile"] = stile, rtile
            bns = small.tile([128, CHT, 6], F32, tag="bns")
            ags = small.tile([128, CHT, 2], F32, tag="ags")
            bnr = small.tile([128, CHT, 6], F32, tag="bnr")
            agr = small.tile([128, CHT, 2], F32, tag="agr")
            for jj in range(CHT):
                nc.vector.bn_stats(bns[:, jj, :], s_raw[:, jj, :])
                nc.vector.bn_stats(bnr[:, jj, :], r_raw[:, jj, :])
            for jj in range(CHT):
                nc.vector.bn_aggr(ags[:, jj, :], bns[:, jj, :])
                nc.vector.bn_aggr(agr[:, jj, :], bnr[:, jj, :])
            sig_s = small.tile([128, CHT], F32, tag="sig_s")
            nc.scalar.activation(out=sig_s[:], in_=ags[:, :, 1], func=AF.Sqrt,
                                 bias=epsln[:, :], scale=1.0)
            sig_r = small.tile([128, CHT], F32, tag="sig_r")
            nc.scalar.activation(out=sig_r[:], in_=agr[:, :, 1], func=AF.Sqrt,
                                 bias=epsln[:, :], scale=1.0)
            st["sig_s"], st["sig_r"] = sig_s, sig_r
            st["mu_s"], st["agr"] = ags, agr
            nc.gpsimd.memset(stile[:, :, WC0], 1.0)

        def emit_compute(ch):
            j0 = ch * CHT
            st = chunk_state.pop(ch)
            s_raw, r_raw = st["s_raw"], st["r_raw"]
            stile, rtile = st["stile"], st["rtile"]
            invs_s = small.tile([128, CHT], F32, tag="invs_s")
            invs_r = small.tile([128, CHT], F32, tag="invs_r")
            nc.vector.reciprocal(out=invs_s[:], in_=st["sig_s"][:])
            nc.vector.reciprocal(out=invs_r[:], in_=st["sig_r"][:])
            nc.vector.scalar_tensor_tensor(
                out=stile[:, :, WC0 + 1], in0=st["mu_s"][:, :, 0],
                scalar=-1.0, op0=AL.mult, op1=AL.mult, in1=invs_s[:])
            nc.vector.scalar_tensor_tensor(
                out=c_col[:, j0:j0 + CHT], in0=st["agr"][:, :, 0], scalar=-1.0,
                op0=AL.mult, op1=AL.mult, in1=invs_r[:])
            nc.scalar.activation(out=stile[:, :, WC0 + 2],
                                 in_=c_col[:, j0:j0 + CHT], func=AF.Copy)
            for jj in range(CHT):
                nc.scalar.activation(
                    out=stile[:, jj, SP0:SP0 + 256], in_=s_raw[:, jj, :],
                    func=AF.Copy, bias=0.0, scale=invs_s[:, jj:jj + 1])
                nc.scalar.activation(
                    out=rtile[:, jj * 256:(jj + 1) * 256],
                    in_=r_raw[:, jj, :], func=AF.Copy, bias=0.0,
                    scale=invs_r[:, jj:jj + 1])
            for jj in range(CHT):
                j = j0 + jj
                fst = (j == 0)
                lst = (j == NT - 1)
                rhs_ws = stile[:, jj, WC0:SP0 + 256]     # [wcol | s'] 259
                rhs_r = rtile[:, jj * 256:(jj + 1) * 256]
                for h in range(2):
                    nc.tensor.matmul(
                        out=(b_rs0 if h == 0 else b_rs1)[:],
                        lhsT=rtile[:, jj * 256 + h * 128:jj * 256 + (h + 1) * 128],
                        rhs=rhs_ws, start=fst, stop=lst)
                for h in range(2):
                    nc.tensor.matmul(
                        out=(b_ss0 if h == 0 else b_ss1)[:],
                        lhsT=stile[:, jj, SP0 + h * 128:SP0 + (h + 1) * 128],
                        rhs=rhs_ws, start=fst, stop=lst)
                for h in range(2):
                    nc.tensor.matmul(
                        out=(b_rr0 if h == 0 else b_rr1)[:],
                        lhsT=rtile[:, jj * 256 + h * 128:jj * 256 + (h + 1) * 128],
                        rhs=rhs_r, start=fst, stop=lst)
                nc.tensor.matmul(out=b_wg[0:3, :],
                                 lhsT=stile[:, jj, WC0:WC0 + 3],
                                 rhs=stile[:, jj, WC0:WC0 + 3],
                                 start=fst, stop=lst)
            nc.sync.dma_start_transpose(rT_all[:, j0:j0 + CHT, :, :],
                                        rtile[:])

        emit_load(0)
        emit_load(1)
        emit_stats(0)
        for ch in range(NCH):
            if ch + 2 < NCH:
                emit_load(ch + 2)
            if ch + 1 < NCH:
                emit_stats(ch + 1)
            emit_compute(ch)

        # ---- evac C matrices (bf16) + S-gram ----
        crs_sb = post.tile([128, 2, 259], BF, tag="crs")
        css_sb = post.tile([128, 2, 259], BF, tag="css")
        crr_sb = post.tile([128, 2, 256], BF, tag="crr")
        sg4_sb = post.tile([4, 3], BF, tag="sg4")
        nc.vector.tensor_scalar(crs_sb[:, 0, :], b_rs0[:], 1.0, None, AL.mult)
        nc.vector.tensor_scalar(crs_sb[:, 1, :], b_rs1[:], 1.0, None, AL.mult)
        nc.scalar.activation(out=css_sb[:, 0, :], in_=b_ss0[:], func=AF.Copy)
        nc.scalar.activation(out=css_sb[:, 1, :], in_=b_ss1[:], func=AF.Copy)
        nc.vector.tensor_scalar(crr_sb[:, 0, :], b_rr0[:], 1.0, None, AL.mult)
        nc.scalar.activation(out=crr_sb[:, 1, :], in_=b_rr1[:], func=AF.Copy)
        nc.vector.memset(sg4_sb[:], 0.0)
        nc.vector.tensor_scalar(sg4_sb[0:3, :], b_wg[0:3, :], 1.0, None,
                                AL.mult)
    # col indices in sgb: Sa=1, Sc=2, Saa=4, Sac=5, Scc=8

    with tc.tile_pool(name="pb", bufs=1, space="PSUM") as pb:
        th4 = pb.tile([128, 4, 3], F32, tag="th4")  # q:[beta|eps|alpha] k:[delta|gam|zeta]
        xh_ps = pb.tile([128, 2, 256], F32, tag="xh")
        g_ps = pb.tile([128, 2, 128], F32, tag="g")
        z_ps = pb.tile([128, 2, 256], F32, tag="z")
        tr_ps = pb.tile([128, 2, 128], BF, tag="tr")
        sgb_ps = pb.tile([128, 3, 3], F32, tag="sgb_ps")
        # S-gram values broadcast to all partitions: 3 indicator matmuls
        for rr2 in range(3):
            nc.tensor.matmul(out=sgb_ps[:, rr2, :],
                             lhsT=sel4[:, rr2 * 128:(rr2 + 1) * 128],
                             rhs=sg4_sb[:, :], start=True, stop=True)
        sgb = post.tile([128, 9], F32, tag="sgb")
        nc.vector.tensor_scalar(sgb[:], sgb_ps[:], 1.0, None, AL.mult)

        for ih in range(2):
            for lh in range(2):
                nc.tensor.matmul(out=th4[:, ih, :],
                                 lhsT=wqsT[:, lh, ih * 128:(ih + 1) * 128],
                                 rhs=css_sb[:, lh, 0:3],
                                 start=(lh == 0), stop=(lh == 1))
                nc.tensor.matmul(out=th4[:, 2 + ih, :],
                                 lhsT=wkrT[:, lh, ih * 128:(ih + 1) * 128],
                                 rhs=crs_sb[:, lh, 0:3],
                                 start=(lh == 0), stop=(lh == 1))
        # Xk_h = Wk_h C_rs   [o in h, j_s(256)]
        for h in range(2):
            for lh in range(2):
                nc.tensor.matmul(out=xh_ps[:, h, :],
                                 lhsT=wkrT[:, lh, h * 128:(h + 1) * 128],
                                 rhs=crs_sb[:, lh, 3:259],
                                 start=(lh == 0), stop=(lh == 1))
        x_sb = post.tile([128, 2, 256], BF, tag="x_sb")
        nc.vector.tensor_scalar(x_sb[:, 0, :], xh_ps[:, 0, :], 1.0, None,
                                AL.mult)
        nc.scalar.activation(out=x_sb[:, 1, :], in_=xh_ps[:, 1, :],
                             func=AF.Copy)
        xT_sb = post.tile([128, 2, 2, 128], BF, tag="xT")
        for h in range(2):
            for jh in range(2):
                nc.tensor.transpose(tr_ps[:, jh, :],
                                    x_sb[:, h, jh * 128:(jh + 1) * 128],
                                    eye_sb[:])
            nc.vector.tensor_scalar(xT_sb[:, h, 0, :], tr_ps[:, 0, :], 1.0,
                                    None, AL.mult)
            nc.scalar.activation(out=xT_sb[:, h, 1, :], in_=tr_ps[:, 1, :],
                                 func=AF.Copy)
        # G_h[i,o] = sum_js WqS[i,js] XkT[js,o]   (rank-1 terms added to
        # the same accumulation group later via K=2 matmuls)
        for h in range(2):
            for jh in range(2):
                nc.tensor.matmul(out=g_ps[:, h, :],
                                 lhsT=wqsT[:, jh, h * 128:(h + 1) * 128],
                                 rhs=xT_sb[:, h, jh, :],
                                 start=(jh == 0), stop=False)
        # d4 = [dq | dk] diag terms
        d4 = small.tile([128, 4], F32, tag="d4")
        dscr = post.tile([128, 256], F32, tag="dscr")
        for a in range(2):
            for lh in range(2):
                nc.tensor.matmul(out=z_ps[:, a, :],
                                 lhsT=wqsT[:, lh, a * 128:(a + 1) * 128],
                                 rhs=css_sb[:, lh, 3:259],
                                 start=(lh == 0), stop=(lh == 1))
        for a in range(2):
            nc.vector.scalar_tensor_tensor(
                out=dscr[:], in0=z_ps[:, a, :], scalar=0.0, op0=AL.bypass,
                op1=AL.mult, in1=wqs[:, a, :], accum_out=d4[:, a:a + 1])
        for a in range(2):
            for lh in range(2):
                nc.tensor.matmul(out=z_ps[:, a, :],
                                 lhsT=wkrT[:, lh, a * 128:(a + 1) * 128],
                                 rhs=crr_sb[:, lh, :],
                                 start=(lh == 0), stop=(lh == 1))
        for a in range(2):
            nc.vector.scalar_tensor_tensor(
                out=dscr[:], in0=z_ps[:, a, :], scalar=0.0, op0=AL.bypass,
                op1=AL.mult, in1=wkr[:, a, :], accum_out=d4[:, 2 + a:3 + a])

        # norms (q and k combined on [128,4]):
        # n = d + 2*th[...,1]*u + 2*th[...,0]*g + sXX*u^2 + 2*sX*u*g + T*g^2
        nqk2 = small.tile([128, 4], F32, tag="nqk2")
        t1 = small.tile([128, 4], F32, tag="t1")
        t2 = small.tile([128, 4], F32, tag="t2")
        sXX4 = small.tile([128, 4], F32, tag="sXX4")
        sX4 = small.tile([128, 4], F32, tag="sX4")
        for cdst, csrc in ((sXX4[:, 0:2], 4), (sXX4[:, 2:4], 8),
                           (sX4[:, 0:2], 1), (sX4[:, 2:4], 2)):
            nc.vector.tensor_scalar(cdst, _bcfree(sgb, csrc, 2), 1.0, None,
                                    AL.mult)
        nc.vector.tensor_tensor(out=t1[:, 0:2], in0=th4[:, 0:2, 1],
                                in1=sqk4[:, 0:2], op=AL.mult)
        nc.vector.tensor_tensor(out=t1[:, 2:4], in0=th4[:, 2:4, 2],
                                in1=sqk4[:, 2:4], op=AL.mult)
        nc.vector.scalar_tensor_tensor(out=nqk2[:], in0=t1[:], scalar=2.0,
                                       op0=AL.mult, op1=AL.add, in1=d4[:])
        nc.vector.tensor_tensor(out=t1[:], in0=th4[:, :, 0], in1=bqk4[:],
                                op=AL.mult)
        nc.vector.scalar_tensor_tensor(out=nqk2[:], in0=t1[:], scalar=2.0,
                                       op0=AL.mult, op1=AL.add, in1=nqk2[:])
        nc.vector.tensor_tensor(out=t1[:], in0=sqk4[:], in1=sqk4[:],
                                op=AL.mult)
        nc.vector.tensor_tensor(out=t2[:], in0=t1[:], in1=sXX4[:], op=AL.mult)
        nc.vector.tensor_tensor(out=nqk2[:], in0=nqk2[:], in1=t2[:], op=AL.add)
        nc.vector.tensor_tensor(out=t1[:], in0=sqk4[:], in1=bqk4[:],
                                op=AL.mult)
        nc.vector.tensor_tensor(out=t2[:], in0=t1[:], in1=sX4[:], op=AL.mult)
        nc.vector.scalar_tensor_tensor(out=nqk2[:], in0=t2[:], scalar=2.0,
                                       op0=AL.mult, op1=AL.add, in1=nqk2[:])
        nc.vector.tensor_tensor(out=t1[:], in0=bqk4[:], in1=bqk4[:],
                                op=AL.mult)
        nc.vector.scalar_tensor_tensor(out=nqk2[:], in0=t1[:], scalar=TCORE,
                                       op0=AL.mult, op1=AL.add, in1=nqk2[:])

        # G rank-1 rows (k-side combos), broadcast fully on-chip
        r12c = small.tile([128, 2, 2], BF, tag="r12c")  # [m(row1/2), h]
        nc.vector.scalar_tensor_tensor(out=r12c[:, 0, :], in0=sqk4[:, 2:4],
                                       scalar=sgb[:, 5:6], op0=AL.mult,
                                       op1=AL.add, in1=th4[:, 2:4, 1])
        nc.vector.scalar_tensor_tensor(out=r12c[:, 0, :], in0=bqk4[:, 2:4],
                                       scalar=sgb[:, 1:2], op0=AL.mult,
                                       op1=AL.add, in1=r12c[:, 0, :])
        nc.vector.scalar_tensor_tensor(out=r12c[:, 1, :], in0=sqk4[:, 2:4],
                                       scalar=sgb[:, 2:3], op0=AL.mult,
                                       op1=AL.add, in1=th4[:, 2:4, 0])
        nc.vector.scalar_tensor_tensor(out=r12c[:, 1, :], in0=bqk4[:, 2:4],
                                       scalar=TCORE, op0=AL.mult,
                                       op1=AL.add, in1=r12c[:, 1, :])
        # per head: pack cols (alpha, beta | r1, r2), transpose to rows,
        # then two K=2 rank-1 matmuls finish the G accumulation group
        abr_sb = post.tile([128, 2, 4], BF, tag="abr")
        for h in range(2):
            nc.vector.tensor_scalar(abr_sb[:, h, 0:1], th4[:, h, 2:3], 1.0,
                                    None, AL.mult)
            nc.vector.tensor_scalar(abr_sb[:, h, 1:2], th4[:, h, 0:1], 1.0,
                                    None, AL.mult)
        nc.vector.tensor_scalar(abr_sb[:, :, 2], r12c[:, 0, :], 1.0, None,
                                AL.mult)
        nc.vector.tensor_scalar(abr_sb[:, :, 3], r12c[:, 1, :], 1.0, None,
                                AL.mult)
        ab_row = post.tile([2, 2, 128], BF, tag="ab_row")
        r12_row = post.tile([2, 2, 128], BF, tag="r12_row")
        for h in range(2):
            nc.tensor.transpose(tr_ps[0:2, 0, :], abr_sb[:, h, 0:2],
                                eye_sb[:])
            nc.tensor.transpose(tr_ps[0:2, 1, :], abr_sb[:, h, 2:4],
                                eye_sb[:])
            nc.scalar.activation(out=ab_row[:, h, :], in_=tr_ps[0:2, 0, :],
                                 func=AF.Copy)
            nc.scalar.activation(out=r12_row[:, h, :], in_=tr_ps[0:2, 1, :],
                                 func=AF.Copy)
        for h in range(2):
            nc.tensor.matmul(out=g_ps[:, h, :], lhsT=ab_row[:, h, :],
                             rhs=skbk_rows[:, h, :], start=False, stop=False)
            nc.tensor.matmul(out=g_ps[:, h, :], lhsT=sqbq_rows[:, h, :],
                             rhs=r12_row[:, h, :], start=False, stop=True)

        # pack [G0 | G1 | nq2 | nk2]  (bf16 collective)
        pack = post.tile([128, 260], BF, tag="pack")
        nc.scalar.activation(out=pack[:, 0:128], in_=g_ps[:, 0, :],
                             func=AF.Copy)
        nc.scalar.activation(out=pack[:, 128:256], in_=g_ps[:, 1, :],
                             func=AF.Copy)
        nc.vector.tensor_scalar(pack[:, 256:260], nqk2[:], 1.0, None, AL.mult)

    cc_in = dram.tile([128, 260], BF)
    cc_out = dram.tile([128, 260], BF)
    nc.sync.dma_start(out=cc_in[:, :], in_=pack[:])
    nc.gpsimd.collective_compute(
        "AllReduce", AL.add,
        replica_groups=[[0, 1], [2, 3], [4, 5], [6, 7]],
        ins=[cc_in.opt()], outs=[cc_out.opt()])

    red = post.tile([128, 260], BF, tag="red")
    nc.sync.dma_start(out=red[:], in_=cc_out[:, :])

    # ================= phase C: softmax + Pp/f assembly ================
    with tc.tile_pool(name="pc", bufs=1, space="PSUM") as pc2:
        e_ps = pc2.tile([128, 2, 2, 128], F32, tag="e_ps")
        # --- PE warmup chain to keep HAM hot across the collective gap ---
        wu_sb = post.tile([128, 128], BF, tag="wu_sb")
        nc.vector.tensor_scalar(wu_sb[:], eye_sb[:], 1.0, None, AL.mult)

        def warmup(n):
            for k in range(n):
                nc.tensor.matmul(out=e_ps[:, 0, 0, :], lhsT=wu_sb[:],
                                 rhs=eye_sb[:], start=True, stop=True)
                nc.scalar.activation(out=wu_sb[:], in_=e_ps[:, 0, 0, :],
                                     func=AF.Copy)

        warmup(18)

        trx_ps = pc2.tile([128, 6, 128], BF, tag="trx")
        tr2_ps = trx_ps[:, 0:2, :]
        invq = small.tile([128, 2], F32, tag="invq")
        invk = small.tile([128, 2], F32, tag="invk")
        for dst, src_off, mul_temp in ((invq, 256, True), (invk, 258, False)):
            sq_ = small.tile([128, 2], F32, tag="invn_sq")
            nc.scalar.activation(out=sq_[:], in_=red[:, src_off:src_off + 2],
                                 func=AF.Sqrt, bias=zb[:, :], scale=1.0)
            nc.vector.tensor_scalar_max(sq_[:], sq_[:], EPS_NORM)
            nc.vector.reciprocal(out=dst[:], in_=sq_[:])
            if mul_temp:
                nc.vector.tensor_tensor(out=dst[:], in0=dst[:],
                                        in1=temp_b[:, :], op=AL.mult)
        invk_bf = small.tile([128, 2], BF, tag="invk_bf")
        nc.vector.tensor_scalar(invk_bf[:], invk[:], 1.0, None, AL.mult)
        nc.tensor.transpose(tr2_ps[0:2, 0, :], invk_bf[:], eye_sb[:])
        ik2_sb = post.tile([2, 128], BF, tag="ik2")
        nc.scalar.activation(out=ik2_sb[:], in_=tr2_ps[0:2, 0, :],
                             func=AF.Copy)
        iktf_ps = pc2.tile([128, 2, 132], F32, tag="iktf_ps")
        ikb_ps = iktf_ps[:, :, 0:128]
        for h in range(2):
            nc.tensor.matmul(out=ikb_ps[:, h, :],
                             lhsT=sel4[0:2, h * 128:(h + 1) * 128],
                             rhs=ik2_sb[:], start=True, stop=True)

        a_sb = post.tile([128, 2, 128], BF, tag="a_sb")
        esc = post.tile([128, 2, 128], F32, tag="esc")
        for h in range(2):
            lh_t = post.tile([128, 128], F32, tag="lh_t")
            nc.vector.tensor_scalar(lh_t[:], red[:, h * 128:(h + 1) * 128],
                                    invq[:, h:h + 1], None, AL.mult)
            nc.vector.tensor_tensor(out=lh_t[:], in0=lh_t[:],
                                    in1=ikb_ps[:, h, :], op=AL.mult)
            rmax = small.tile([128, 1], F32, tag="rmax")
            nc.vector.tensor_reduce(out=rmax[:], in_=lh_t[:], op=AL.max,
                                    axis=AX.X)
            nc.vector.tensor_scalar(rmax[:], rmax[:], -1.0, None, AL.mult)
            rsum = small.tile([128, 1], F32, tag="rsum")
            nc.scalar.activation(out=esc[:, h, :], in_=lh_t[:], func=AF.Exp,
                                 bias=rmax[:, :], scale=1.0,
                                 accum_out=rsum[:])
            nc.vector.reciprocal(out=rsum[:], in_=rsum[:])
            nc.vector.tensor_scalar(a_sb[:, h, :], esc[:, h, :],
                                    rsum[:, :], None, AL.mult)

        attnT = post.tile([128, 2, 128], BF, tag="attnT")
        for h in range(2):
            nc.tensor.transpose(tr2_ps[:, h, :], a_sb[:, h, :], eye_sb[:])
        for h in range(2):
            nc.scalar.activation(out=attnT[:, h, :], in_=tr2_ps[:, h, :],
                                 func=AF.Copy)

        for h in range(2):
            for ph in range(2):
                nc.tensor.matmul(out=e_ps[:, ph, h, :],
                                 lhsT=woT[:, h, ph * 128:(ph + 1) * 128],
                                 rhs=a_sb[:, h, :], start=True, stop=True)
        e_sb = post.tile([128, 2, 2, 128], BF, tag="e_sb")
        for ph in range(2):
            nc.vector.tensor_scalar(e_sb[:, ph, 0, :], e_ps[:, ph, 0, :],
                                    1.0, None, AL.mult)
            nc.scalar.activation(out=e_sb[:, ph, 1, :], in_=e_ps[:, ph, 1, :],
                                 func=AF.Copy)
        et_ps = trx_ps[:, 2:6, :]
        eT_sb = post.tile([128, 2, 256], BF, tag="eT")
        for h in range(2):
            for ph in range(2):
                nc.tensor.transpose(et_ps[:, h * 2 + ph, :],
                                    e_sb[:, ph, h, :], eye_sb[:])
        for h in range(2):
            nc.vector.tensor_scalar(eT_sb[:, h, 0:128],
                                    et_ps[:, h * 2 + 0, :],
                                    1.0, None, AL.mult)
            nc.scalar.activation(out=eT_sb[:, h, 128:256],
                                 in_=et_ps[:, h * 2 + 1, :], func=AF.Copy)
        ppt_ps = pc2.tile([128, 2, 256], F32, tag="ppt")
        for mh in range(2):
            for h in range(2):
                nc.tensor.matmul(out=ppt_ps[:, mh, :],
                                 lhsT=wvr[:, h, mh * 128:(mh + 1) * 128],
                                 rhs=eT_sb[:, h, :],
                                 start=(h == 0), stop=(h == 1))
        pptT = post.tile([128, 2, 256], BF, tag="pptT")
        nc.vector.tensor_scalar(pptT[:, 0, :], ppt_ps[:, 0, :], 1.0, None,
                                AL.mult)
        nc.scalar.activation(out=pptT[:, 1, :], in_=ppt_ps[:, 1, :],
                             func=AF.Copy)

        t_ps = iktf_ps[:, :, 128:130]
        f12_ps = iktf_ps[:, :, 130:132]
        for h in range(2):
            nc.tensor.matmul(out=t_ps[:, h, :], lhsT=attnT[:, h, :],
                             rhs=svbv[:, h, :], start=True, stop=True)
        t_sb = post.tile([128, 2, 2], BF, tag="t_sb")
        nc.vector.tensor_scalar(t_sb[:], t_ps[:, :, :], 1.0, None, AL.mult)
        for ph in range(2):
            for h in range(2):
                nc.tensor.matmul(out=f12_ps[:, ph, :],
                                 lhsT=woT[:, h, ph * 128:(ph + 1) * 128],
                                 rhs=t_sb[:, h, :],
                                 start=(h == 0), stop=(h == 1))
        f12_sb = post.tile([128, 2, 2], BF, tag="f12sb")
        nc.vector.tensor_scalar(f12_sb[:, :, 0], f12_ps[:, :, 0], 1.0, None,
                                AL.mult)
        nc.vector.tensor_tensor(out=f12_sb[:, :, 1], in0=f12_ps[:, :, 1],
                                in1=bo_col[:, :], op=AL.add)
        # f rows: transpose [128,(ph,m)] -> [4,128] (k=ph*2+m), then
        # indicator-MM broadcasts; f2 becomes a K=4 block rhs for phase D
        nc.tensor.transpose(tr2_ps[0:4, 0, :], f12_sb[:, :, :], eye_sb[:])
        f4_sb = post.tile([4, 128], BF, tag="f4_sb")
        nc.scalar.activation(out=f4_sb[:], in_=tr2_ps[0:4, 0, :], func=AF.Copy)
        for ph in range(2):
            nc.tensor.matmul(out=ikb_ps[:, ph, :],
                             lhsT=sel4[:, (ph * 2) * 128:(ph * 2 + 1) * 128],
                             rhs=f4_sb[:], start=True, stop=True)
        f1b = post.tile([128, 256], F32, tag="f1b")
        nc.vector.tensor_scalar(f1b[:], ikb_ps[:, :, :], 1.0, None, AL.mult)
        f24_sb = post.tile([128, 256], BF, tag="f24")
        nc.vector.memset(f24_sb[:], 0.0)
        nc.vector.tensor_tensor(out=f24_sb[0:4, 0:128], in0=f4_sb[:],
                                in1=fm0[:], op=AL.mult)
        nc.vector.tensor_tensor(out=f24_sb[0:4, 128:256], in0=f4_sb[:],
                                in1=fm1[:], op=AL.mult)

        # ============= phase D: output pass (same psum pool) =============
        opsum0 = pc2.tile([128, 256], F32, tag="opsum0")
        opsum1 = pc2.tile([128, 256], F32, tag="opsum1")
        opsum2 = pc2.tile([128, 256], F32, tag="opsum2")
        opsum3 = pc2.tile([128, 256], F32, tag="opsum3")
        op_t = [opsum0, opsum1, opsum2, opsum3]
        for g in range(NT // 4):
            j0 = g * 4
            out_sb = outp.tile([128, 4, 256], BF, tag="out_sb")
            for jj in range(4):
                j = j0 + jj
                opsum = op_t[j % 4]
                nc.tensor.matmul(out=opsum[:], lhsT=rT_all[:, j, 0, :],
                                 rhs=pptT[:, 0, :], start=True, stop=False)
                nc.tensor.matmul(out=opsum[:], lhsT=rT_all[:, j, 1, :],
                                 rhs=pptT[:, 1, :], start=False, stop=False)
                nc.tensor.matmul(out=opsum[:], lhsT=m24[:, :],
                                 rhs=f24_sb[:, :], start=False, stop=True)
                nc.vector.scalar_tensor_tensor(
                    out=out_sb[:, jj, :], in0=f1b[:],
                    scalar=c_col[:, j:j + 1], op0=AL.mult, op1=AL.add,
                    in1=opsum[:])
            nc.sync.dma_start(out=outv[:, j0:j0 + 4, :], in_=out_sb[:])


def _bcfree(tile_, col, n):
    """AP reading tile_[:, col] broadcast n times along free (0-stride)."""
    ap = tile_[:, col:col + 1]
    return bass.AP(tensor=ap.tensor, offset=ap.offset,
                   ap=[list(ap.ap[0])] + [[0, n]])


# ======================= host side =======================

def _sel4():
    s = np.zeros((4, 512), np.float32)
    for k in range(4):
        s[k, k * 128:(k + 1) * 128] = 1.0
    return s.astype(BF16)


def _m24():
    m = np.zeros((128, 128), np.float32)
    m[1, :] = 1.0
    m[3, :] = 1.0
    return m.astype(BF16)


def _prep_shared(inputs):
    f32 = np.float32
    Wq = np.asarray(inputs["Wq"], f32)
    bq = np.asarray(inputs["bq"], f32)
    Wkv = np.asarray(inputs["Wkv"], f32)
    bkv = np.asarray(inputs["bkv"], f32)
    Wo = np.asarray(inputs["Wo"], f32)
    bo = np.asarray(inputs["bo"], f32)
    lnS_w = np.asarray(inputs["lnS_w"], f32)
    lnS_b = np.asarray(inputs["lnS_b"], f32)
    lnR_w = np.asarray(inputs["lnR_w"], f32)
    lnR_b = np.asarray(inputs["lnR_b"], f32)
    temp = np.asarray(inputs["temperature"], f32).reshape(H)

    Wk, Wv = Wkv[:D], Wkv[D:]
    WqS = Wq * lnS_w[None, :]
    WkR = Wk * lnR_w[None, :]
    WvR = Wv * lnR_w[None, :]
    sq = WqS.sum(1)
    sk = WkR.sum(1)
    sv = WvR.sum(1)
    bq2 = Wq @ lnS_b + bq
    bk2 = Wk @ lnR_b + bkv[:D]
    bv2 = Wv @ lnR_b + bkv[D:]

    def halved(M):  # [256, X] -> [128, 2, X] rows split into halves
        return np.ascontiguousarray(
            M.reshape(2, 128, M.shape[1]).transpose(1, 0, 2)).astype(BF16)

    def colh(v):
        return np.ascontiguousarray(v.reshape(2, 128).T, f32)

    svbv = np.stack([sv, bv2], 1)  # [256, 2]
    sqk4 = np.concatenate([colh(sq), colh(sk)], 1)
    bqk4 = np.concatenate([colh(bq2), colh(bk2)], 1)
    return {
        "wqsT": halved(np.ascontiguousarray(WqS.T)),
        "wkrT": halved(np.ascontiguousarray(WkR.T)),
        "wqs": halved(WqS),
        "wkr": halved(WkR),
        "wvr": halved(WvR),
        "woT": halved(np.ascontiguousarray(Wo.T)),
        "svbv": halved(svbv),
        "sqk4": np.ascontiguousarray(sqk4),
        "bqk4": np.ascontiguousarray(bqk4),
        "bo_col": colh(bo),
        "skbk": np.stack([sk.reshape(2, 128),
                          bk2.reshape(2, 128)], 0).astype(BF16),
        "sqbq": np.stack([sq.reshape(2, 128),
                          bq2.reshape(2, 128)], 0).astype(BF16),
        "eye": np.eye(128).astype(BF16),
        "sel4": _sel4(),
        "m24": _m24(),
        "temp": temp.reshape(1, H).astype(f32),
    }


def _get_nc():
    if "nc" not in _nc_cache:
        _nc_cache["nc"] = _build_nc()
    return _nc_cache["nc"]


def run(inputs, trace=False):
    nc = _get_nc()
    shared = _prep_shared(inputs)
    iR = np.asarray(inputs["input_R"], np.float32)
    iS = np.asarray(inputs["input_S"], np.float32)
    in_maps = []
    for ci in range(N_CORES):
        b, half = ci // 2, ci % 2
        m = dict(shared)
        m["x_r"] = np.ascontiguousarray(iR[b, half * T:(half + 1) * T])
        m["x_s"] = np.ascontiguousarray(iS[b, half * T:(half + 1) * T])
        in_maps.append(m)
    res = run_bass_kernel_spmd(nc, in_maps, list(range(N_CORES)), trace=trace)
    out = np.zeros((B, N, D), np.float32)
    for ci in range(N_CORES):
        b, half = ci // 2, ci % 2
        out[b, half * T:(half + 1) * T] = np.asarray(
            res.results[ci]["out"]).astype(np.float32)
    return out, res


def kernel(**inputs):
    out, _ = run(inputs, trace=False)
    return out


# revision 33
# speedup vs baseline: 1.2060x; 1.0497x over previous
"""Channel-attention (XCA-style) Trainium2 kernel, 8-way SPMD — v3.

Shapes (hardcoded): B=4, N=16384, D=256, H=2 heads, c=128.
Sharding: core ci -> batch b=ci//2, token half ci%2 (T=8192 tokens/core).

C-matrix factorization: accumulate token-contraction Grams of the scaled
raw inputs (C_rs, C_ss, C_rr + moment vectors against [1, a_t, c_t]),
then assemble attention logits G and the q/k L2 norms as small [256x256]
weight products, pair-AllReduce only [128,260] f32, and collapse the
whole v/attn@v/Wo path into one matrix Pp applied per token from the
d-major transpose of the scaled kv input.

v3: chunk-level DMA transposes, per-chunk stats tiles + deep buffering
for pipelining, batched DRAM bounces, PE warmup chain across the
collective gap.
"""
import sys, types

sys.path.insert(0, "/opt/trn_rl_repo")

try:
    import antenv
    if "antenv.axon_hooks" not in sys.modules:
        _hooks = types.ModuleType("antenv.axon_hooks")
        _hooks._hook = None
        _hooks.set_axon_ntff_profile_hook = lambda h: setattr(_hooks, "_hook", h)
        _hooks.get_axon_ntff_profile_hook = lambda: _hooks._hook
        sys.modules["antenv.axon_hooks"] = _hooks
        antenv.axon_hooks = _hooks
        from trn_agent_boot.trn_boot import _ntff_profile_via_ctypes
        _hooks.set_axon_ntff_profile_hook(
            _ntff_profile_via_ctypes("/opt/axon/libaxon_pjrt.so"))
except Exception:
    pass

import numpy as np
import ml_dtypes

import concourse.bass as bass
import concourse.bacc as bacc
import concourse.mybir as mybir
import concourse.tile as tile
from concourse.bass_utils import run_bass_kernel_spmd

BF16 = ml_dtypes.bfloat16
F32 = mybir.dt.float32
BF = mybir.dt.bfloat16
AL = mybir.AluOpType
AF = mybir.ActivationFunctionType
AX = mybir.AxisListType

B, N, D, H = 4, 16384, 256, 2
T = N // 2                  # tokens per core
NT = 64                     # token tiles per core (inner j), token = p*64 + j
CHT = 8                     # tiles per chunk
NCH = NT // CHT             # 8 chunks
EPS_LN = 1e-5
EPS_NORM = 1e-12
N_CORES = 8
TCORE = float(T)

# stile column layout (bf16): [pad 0:13 | wcol 13:16 | s' 16:272]
WC0 = 13
SP0 = 16
SW = 272

_nc_cache = {}


def _bcast(ap, rows=128):
    return bass.AP(tensor=ap.tensor, offset=ap.offset,
                   ap=[[0, rows]] + [list(x) for x in ap.ap[1:]])


def _build_nc():
    nc = bacc.Bacc("TRN2", target_bir_lowering=False, debug=False,
                   num_devices=N_CORES)

    def ein(name, shape, dt=F32):
        return nc.dram_tensor(name, list(shape), dt, kind="ExternalInput")

    d_s = ein("x_s", [T, D])            # q source shard (input_S)
    d_r = ein("x_r", [T, D])            # kv source shard (input_R)
    d_wqsT = ein("wqsT", [128, 2, D], BF)
    d_wkrT = ein("wkrT", [128, 2, D], BF)
    d_wqs = ein("wqs", [128, 2, D], BF)
    d_wkr = ein("wkr", [128, 2, D], BF)
    d_wvr = ein("wvr", [128, 2, D], BF)
    d_woT = ein("woT", [128, 2, D], BF)
    d_svbv = ein("svbv", [128, 2, 2], BF)
    d_sqk4 = ein("sqk4", [128, 4])      # [sq_h0 sq_h1 sk_h0 sk_h1]
    d_bqk4 = ein("bqk4", [128, 4])      # [bq2 | bk2]
    d_boc = ein("bo_col", [128, 2])
    d_skbk = ein("skbk", [2, 2, 128], BF)
    d_sqbq = ein("sqbq", [2, 2, 128], BF)
    d_eye = ein("eye", [128, 128], BF)
    d_sel4 = ein("sel4", [4, 512], BF)
    d_m24 = ein("m24", [128, 128], BF)
    d_temp = ein("temp", [1, 2])
    d_out = nc.dram_tensor("out", [T, D], BF, kind="ExternalOutput")

    svw = d_s.rearrange("(p j) d -> p j d", p=128)
    rvw = d_r.rearrange("(p j) d -> p j d", p=128)
    outv = d_out.rearrange("(p j) d -> p j d", p=128)

    with tile.TileContext(nc) as tc:
        import contextlib
        with contextlib.ExitStack() as ctx:
            _body(ctx, tc, nc, svw, rvw, outv, d_wqsT, d_wkrT, d_wqs, d_wkr,
                  d_wvr, d_woT, d_svbv, d_sqk4, d_bqk4, d_boc, d_skbk, d_sqbq,
                  d_eye, d_sel4, d_m24, d_temp)
    nc.finalize()
    return nc


def _body(ctx, tc, nc, svw, rvw, outv, d_wqsT, d_wkrT, d_wqs, d_wkr, d_wvr,
          d_woT, d_svbv, d_sqk4, d_bqk4, d_boc, d_skbk, d_sqbq, d_eye, d_sel4, d_m24,
          d_temp):
    E = ctx.enter_context
    consts = E(tc.tile_pool(name="consts", bufs=1))
    stats = E(tc.tile_pool(name="stats", bufs=1))
    ldp = E(tc.tile_pool(name="ldp", bufs=4))
    cbp = E(tc.tile_pool(name="cbp", bufs=3))
    small = E(tc.tile_pool(name="small", bufs=4))
    pers = E(tc.tile_pool(name="pers", bufs=1))
    post = E(tc.tile_pool(name="post", bufs=1))
    outp = E(tc.tile_pool(name="outp", bufs=2))
    dram = E(tc.tile_pool(name="dram", bufs=1, space="DRAM"))

    # ---------------- constants ----------------
    wqsT = consts.tile([128, 2, D], BF, tag="wqsT")
    wkrT = consts.tile([128, 2, D], BF, tag="wkrT")
    wqs = consts.tile([128, 2, D], BF, tag="wqs")
    wkr = consts.tile([128, 2, D], BF, tag="wkr")
    wvr = consts.tile([128, 2, D], BF, tag="wvr")
    woT = consts.tile([128, 2, D], BF, tag="woT")
    for dst, src in ((wqsT, d_wqsT), (wkrT, d_wkrT), (wqs, d_wqs),
                     (wkr, d_wkr), (wvr, d_wvr), (woT, d_woT)):
        nc.sync.dma_start(out=dst[:], in_=src[:, :, :])
    svbv = consts.tile([128, 2, 2], BF, tag="svbv")
    nc.sync.dma_start(out=svbv[:], in_=d_svbv[:, :, :])
    sqk4 = consts.tile([128, 4], F32, tag="sqk4")
    bqk4 = consts.tile([128, 4], F32, tag="bqk4")
    bo_col = consts.tile([128, 2], F32, tag="boc")
    for dst, src in ((sqk4, d_sqk4), (bqk4, d_bqk4), (bo_col, d_boc)):
        nc.sync.dma_start(out=dst[:], in_=src[:, :])
    skbk_rows = consts.tile([2, 2, 128], BF, tag="skbk")
    sqbq_rows = consts.tile([2, 2, 128], BF, tag="sqbq")
    nc.sync.dma_start(out=skbk_rows[:], in_=d_skbk[:, :, :])
    nc.sync.dma_start(out=sqbq_rows[:], in_=d_sqbq[:, :, :])
    eye_sb = consts.tile([128, 128], BF, tag="eye")
    nc.sync.dma_start(out=eye_sb[:], in_=d_eye[:, :])
    sel4 = consts.tile([4, 512], BF, tag="sel4")
    nc.sync.dma_start(out=sel4[:], in_=d_sel4[:, :])
    m24 = consts.tile([128, 128], BF, tag="m24")
    nc.sync.dma_start(out=m24[:], in_=d_m24[:, :])
    ones4 = consts.tile([4, 128], BF, tag="ones4")
    nc.vector.memset(ones4[:], 1.0)
    fm0 = consts.tile([4, 128], BF, tag="fm0")
    fm1 = consts.tile([4, 128], BF, tag="fm1")
    nc.vector.tensor_scalar(fm0[:], sel4[:, 128:256], 1.0, None, AL.mult)
    nc.vector.tensor_scalar(fm1[:], sel4[:, 384:512], 1.0, None, AL.mult)
    temp_b = consts.tile([128, 2], F32, tag="tempb")
    nc.sync.dma_start(out=temp_b[:], in_=_bcast(d_temp[:, :]))
    ones_row = consts.tile([1, 128], BF, tag="ones")
    nc.vector.memset(ones_row[:], 1.0)
    epsln = consts.tile([128, 1], F32, tag="epsln")
    nc.vector.memset(epsln[:], EPS_LN)
    zb = consts.tile([128, 1], F32, tag="zb")
    nc.vector.memset(zb[:], 0.0)

    c_col = stats.tile([128, NT], F32, tag="c_col")
    sqscr = stats.tile([128, 256], BF, tag="sqscr")   # ACT square scratch
    sqscr2 = stats.tile([128, 256], BF, tag="sqscr2")  # DVE square scratch
    rT_all = pers.tile([128, NT, 2, 128], BF, tag="rT")

    # ================= phase A: stream chunks =================
    with tc.tile_pool(name="accA", bufs=1, space="PSUM") as accA:
        b_rs0 = accA.tile([128, 259], F32, tag="b_rs0")
        b_rs1 = accA.tile([128, 259], F32, tag="b_rs1")
        b_ss0 = accA.tile([128, 259], F32, tag="b_ss0")
        b_ss1 = accA.tile([128, 259], F32, tag="b_ss1")
        b_rr0 = accA.tile([128, 256], F32, tag="b_rr0")
        b_rr1 = accA.tile([128, 256], F32, tag="b_rr1")
        b_wg = accA.tile([128, 3], F32, tag="b_wg")

        chunk_state = {}

        def emit_load(ch):
            j0 = ch * CHT
            s_raw = ldp.tile([128, CHT, D], BF, tag="s_raw")
            r_raw = ldp.tile([128, CHT, D], BF, tag="r_raw")
            nc.gpsimd.dma_start(out=s_raw[:], in_=svw[:, j0:j0 + CHT, :])
            nc.gpsimd.dma_start(out=r_raw[:], in_=rvw[:, j0:j0 + CHT, :])
            chunk_state[ch] = {"s_raw": s_raw, "r_raw": r_raw}

        def emit_stats(ch):
            j0 = ch * CHT
            st = chunk_state[ch]
            s_raw, r_raw = st["s_raw"], st["r_raw"]
            stile = cbp.tile([128, CHT, SW], BF, tag="stile")
            rtile = cbp.tile([128, CHT * 256], BF, tag="rtile")
            st["stile"], st["rtile"] = stile, rtile
            bns = small.tile([128, CHT, 6], F32, tag="bns")
            ags = small.tile([128, CHT, 2], F32, tag="ags")
            bnr = small.tile([128, CHT, 6], F32, tag="bnr")
            agr = small.tile([128, CHT, 2], F32, tag="agr")
            for jj in range(CHT):
                nc.vector.bn_stats(bns[:, jj, :], s_raw[:, jj, :])
                nc.vector.bn_stats(bnr[:, jj, :], r_raw[:, jj, :])
            for jj in range(CHT):
                nc.vector.bn_aggr(ags[:, jj, :], bns[:, jj, :])
                nc.vector.bn_aggr(agr[:, jj, :], bnr[:, jj, :])
            sig_s = small.tile([128, CHT], F32, tag="sig_s")
            nc.scalar.activation(out=sig_s[:], in_=ags[:, :, 1], func=AF.Sqrt,
                                 bias=epsln[:, :], scale=1.0)
            sig_r = small.tile([128, CHT], F32, tag="sig_r")
            nc.scalar.activation(out=sig_r[:], in_=agr[:, :, 1], func=AF.Sqrt,
                                 bias=epsln[:, :], scale=1.0)
            st["sig_s"], st["sig_r"] = sig_s, sig_r
            st["mu_s"], st["agr"] = ags, agr
            nc.gpsimd.memset(stile[:, :, WC0], 1.0)

        def emit_compute(ch):
            j0 = ch * CHT
            st = chunk_state.pop(ch)
            s_raw, r_raw = st["s_raw"], st["r_raw"]
            stile, rtile = st["stile"], st["rtile"]
            invs_s = small.tile([128, CHT], F32, tag="invs_s")
            invs_r = small.tile([128, CHT], F32, tag="invs_r")
            nc.vector.reciprocal(out=invs_s[:], in_=st["sig_s"][:])
            nc.vector.reciprocal(out=invs_r[:], in_=st["sig_r"][:])
            nc.vector.scalar_tensor_tensor(
                out=stile[:, :, WC0 + 1], in0=st["mu_s"][:, :, 0],
                scalar=-1.0, op0=AL.mult, op1=AL.mult, in1=invs_s[:])
            nc.vector.scalar_tensor_tensor(
                out=c_col[:, j0:j0 + CHT], in0=st["agr"][:, :, 0], scalar=-1.0,
                op0=AL.mult, op1=AL.mult, in1=invs_r[:])
            nc.scalar.activation(out=stile[:, :, WC0 + 2],
                                 in_=c_col[:, j0:j0 + CHT], func=AF.Copy)
            for jj in range(CHT):
                nc.scalar.activation(
                    out=stile[:, jj, SP0:SP0 + 256], in_=s_raw[:, jj, :],
                    func=AF.Copy, bias=0.0, scale=invs_s[:, jj:jj + 1])
                nc.scalar.activation(
                    out=rtile[:, jj * 256:(jj + 1) * 256],
                    in_=r_raw[:, jj, :], func=AF.Copy, bias=0.0,
                    scale=invs_r[:, jj:jj + 1])
            for jj in range(CHT):
                j = j0 + jj
                fst = (j == 0)
                lst = (j == NT - 1)
                rhs_ws = stile[:, jj, WC0:SP0 + 256]     # [wcol | s'] 259
                rhs_r = rtile[:, jj * 256:(jj + 1) * 256]
                for h in range(2):
                    nc.tensor.matmul(
                        out=(b_rs0 if h == 0 else b_rs1)[:],
                        lhsT=rtile[:, jj * 256 + h * 128:jj * 256 + (h + 1) * 128],
                        rhs=rhs_ws, start=fst, stop=lst)
                for h in range(2):
                    nc.tensor.matmul(
                        out=(b_ss0 if h == 0 else b_ss1)[:],
                        lhsT=stile[:, jj, SP0 + h * 128:SP0 + (h + 1) * 128],
                        rhs=rhs_ws, start=fst, stop=lst)
                for h in range(2):
                    nc.tensor.matmul(
                        out=(b_rr0 if h == 0 else b_rr1)[:],
                        lhsT=rtile[:, jj * 256 + h * 128:jj * 256 + (h + 1) * 128],
                        rhs=rhs_r, start=fst, stop=lst)
                nc.tensor.matmul(out=b_wg[0:3, :],
                                 lhsT=stile[:, jj, WC0:WC0 + 3],
                                 rhs=stile[:, jj, WC0:WC0 + 3],
                                 start=fst, stop=lst)
            nc.sync.dma_start_transpose(rT_all[:, j0:j0 + CHT, :, :],
                                        rtile[:])

        emit_load(0)
        emit_load(1)
        emit_stats(0)
        for ch in range(NCH):
            if ch + 2 < NCH:
                emit_load(ch + 2)
            if ch + 1 < NCH:
                emit_stats(ch + 1)
            emit_compute(ch)

        # ---- evac C matrices (bf16) + S-gram ----
        crs_sb = post.tile([128, 2, 259], BF, tag="crs")
        css_sb = post.tile([128, 2, 259], BF, tag="css")
        crr_sb = post.tile([128, 2, 256], BF, tag="crr")
        sg4_sb = post.tile([4, 3], BF, tag="sg4")
        nc.vector.tensor_scalar(crs_sb[:, 0, :], b_rs0[:], 1.0, None, AL.mult)
        nc.vector.tensor_scalar(crs_sb[:, 1, :], b_rs1[:], 1.0, None, AL.mult)
        nc.scalar.activation(out=css_sb[:, 0, :], in_=b_ss0[:], func=AF.Copy)
        nc.scalar.activation(out=css_sb[:, 1, :], in_=b_ss1[:], func=AF.Copy)
        nc.vector.tensor_scalar(crr_sb[:, 0, :], b_rr0[:], 1.0, None, AL.mult)
        nc.scalar.activation(out=crr_sb[:, 1, :], in_=b_rr1[:], func=AF.Copy)
        nc.vector.memset(sg4_sb[:], 0.0)
        nc.vector.tensor_scalar(sg4_sb[0:3, :], b_wg[0:3, :], 1.0, None,
                                AL.mult)
    # col indices in sgb: Sa=1, Sc=2, Saa=4, Sac=5, Scc=8

    with tc.tile_pool(name="pb", bufs=1, space="PSUM") as pb:
        th4 = pb.tile([128, 4, 3], F32, tag="th4")  # q:[beta|eps|alpha] k:[delta|gam|zeta]
        xh_ps = pb.tile([128, 2, 256], F32, tag="xh")
        g_ps = pb.tile([128, 2, 128], F32, tag="g")
        z_ps = pb.tile([128, 2, 256], F32, tag="z")
        tr_ps = pb.tile([128, 2, 128], BF, tag="tr")
        sgb_ps = pb.tile([128, 3, 3], F32, tag="sgb_ps")
        # S-gram values broadcast to all partitions: 3 indicator matmuls
        for rr2 in range(3):
            nc.tensor.matmul(out=sgb_ps[:, rr2, :],
                             lhsT=sel4[:, rr2 * 128:(rr2 + 1) * 128],
                             rhs=sg4_sb[:, :], start=True, stop=True)
        sgb = post.tile([128, 9], F32, tag="sgb")
        nc.vector.tensor_scalar(sgb[:], sgb_ps[:], 1.0, None, AL.mult)

        for ih in range(2):
            for lh in range(2):
                nc.tensor.matmul(out=th4[:, ih, :],
                                 lhsT=wqsT[:, lh, ih * 128:(ih + 1) * 128],
                                 rhs=css_sb[:, lh, 0:3],
                                 start=(lh == 0), stop=(lh == 1))
                nc.tensor.matmul(out=th4[:, 2 + ih, :],
                                 lhsT=wkrT[:, lh, ih * 128:(ih + 1) * 128],
                                 rhs=crs_sb[:, lh, 0:3],
                                 start=(lh == 0), stop=(lh == 1))
        # Xk_h = Wk_h C_rs   [o in h, j_s(256)]
        for h in range(2):
            for lh in range(2):
                nc.tensor.matmul(out=xh_ps[:, h, :],
                                 lhsT=wkrT[:, lh, h * 128:(h + 1) * 128],
                                 rhs=crs_sb[:, lh, 3:259],
                                 start=(lh == 0), stop=(lh == 1))
        x_sb = post.tile([128, 2, 256], BF, tag="x_sb")
        nc.vector.tensor_scalar(x_sb[:, 0, :], xh_ps[:, 0, :], 1.0, None,
                                AL.mult)
        nc.scalar.activation(out=x_sb[:, 1, :], in_=xh_ps[:, 1, :],
                             func=AF.Copy)
        xT_sb = post.tile([128, 2, 2, 128], BF, tag="xT")
        for h in range(2):
            for jh in range(2):
                nc.tensor.transpose(tr_ps[:, jh, :],
                                    x_sb[:, h, jh * 128:(jh + 1) * 128],
                                    eye_sb[:])
            nc.vector.tensor_scalar(xT_sb[:, h, 0, :], tr_ps[:, 0, :], 1.0,
                                    None, AL.mult)
            nc.scalar.activation(out=xT_sb[:, h, 1, :], in_=tr_ps[:, 1, :],
                                 func=AF.Copy)
        # G_h[i,o] = sum_js WqS[i,js] XkT[js,o]   (rank-1 terms added to
        # the same accumulation group later via K=2 matmuls)
        for h in range(2):
            for jh in range(2):
                nc.tensor.matmul(out=g_ps[:, h, :],
                                 lhsT=wqsT[:, jh, h * 128:(h + 1) * 128],
                                 rhs=xT_sb[:, h, jh, :],
                                 start=(jh == 0), stop=False)
        # d4 = [dq | dk] diag terms
        d4 = small.tile([128, 4], F32, tag="d4")
        dscr = post.tile([128, 256], F32, tag="dscr")
        for a in range(2):
            for lh in range(2):
                nc.tensor.matmul(out=z_ps[:, a, :],
                                 lhsT=wqsT[:, lh, a * 128:(a + 1) * 128],
                                 rhs=css_sb[:, lh, 3:259],
                                 start=(lh == 0), stop=(lh == 1))
        for a in range(2):
            nc.vector.scalar_tensor_tensor(
                out=dscr[:], in0=z_ps[:, a, :], scalar=0.0, op0=AL.bypass,
                op1=AL.mult, in1=wqs[:, a, :], accum_out=d4[:, a:a + 1])
        for a in range(2):
            for lh in range(2):
                nc.tensor.matmul(out=z_ps[:, a, :],
                                 lhsT=wkrT[:, lh, a * 128:(a + 1) * 128],
                                 rhs=crr_sb[:, lh, :],
                                 start=(lh == 0), stop=(lh == 1))
        for a in range(2):
            nc.vector.scalar_tensor_tensor(
                out=dscr[:], in0=z_ps[:, a, :], scalar=0.0, op0=AL.bypass,
                op1=AL.mult, in1=wkr[:, a, :], accum_out=d4[:, 2 + a:3 + a])

        # norms (q and k combined on [128,4]):
        # n = d + 2*th[...,1]*u + 2*th[...,0]*g + sXX*u^2 + 2*sX*u*g + T*g^2
        nqk2 = small.tile([128, 4], F32, tag="nqk2")
        t1 = small.tile([128, 4], F32, tag="t1")
        t2 = small.tile([128, 4], F32, tag="t2")
        sXX4 = small.tile([128, 4], F32, tag="sXX4")
        sX4 = small.tile([128, 4], F32, tag="sX4")
        for cdst, csrc in ((sXX4[:, 0:2], 4), (sXX4[:, 2:4], 8),
                           (sX4[:, 0:2], 1), (sX4[:, 2:4], 2)):
            nc.vector.tensor_scalar(cdst, _bcfree(sgb, csrc, 2), 1.0, None,
                                    AL.mult)
        nc.vector.tensor_tensor(out=t1[:, 0:2], in0=th4[:, 0:2, 1],
                                in1=sqk4[:, 0:2], op=AL.mult)
        nc.vector.tensor_tensor(out=t1[:, 2:4], in0=th4[:, 2:4, 2],
                                in1=sqk4[:, 2:4], op=AL.mult)
        nc.vector.scalar_tensor_tensor(out=nqk2[:], in0=t1[:], scalar=2.0,
                                       op0=AL.mult, op1=AL.add, in1=d4[:])
        nc.vector.tensor_tensor(out=t1[:], in0=th4[:, :, 0], in1=bqk4[:],
                                op=AL.mult)
        nc.vector.scalar_tensor_tensor(out=nqk2[:], in0=t1[:], scalar=2.0,
                                       op0=AL.mult, op1=AL.add, in1=nqk2[:])
        nc.vector.tensor_tensor(out=t1[:], in0=sqk4[:], in1=sqk4[:],
                                op=AL.mult)
        nc.vector.tensor_tensor(out=t2[:], in0=t1[:], in1=sXX4[:], op=AL.mult)
        nc.vector.tensor_tensor(out=nqk2[:], in0=nqk2[:], in1=t2[:], op=AL.add)
        nc.vector.tensor_tensor(out=t1[:], in0=sqk4[:], in1=bqk4[:],
                                op=AL.mult)
        nc.vector.tensor_tensor(out=t2[:], in0=t1[:], in1=sX4[:], op=AL.mult)
        nc.vector.scalar_tensor_tensor(out=nqk2[:], in0=t2[:], scalar=2.0,
                                       op0=AL.mult, op1=AL.add, in1=nqk2[:])
        nc.vector.tensor_tensor(out=t1[:], in0=bqk4[:], in1=bqk4[:],
                                op=AL.mult)
        nc.vector.scalar_tensor_tensor(out=nqk2[:], in0=t1[:], scalar=TCORE,
                                       op0=AL.mult, op1=AL.add, in1=nqk2[:])

        # G rank-1 rows (k-side combos), broadcast fully on-chip
        r12c = small.tile([128, 2, 2], BF, tag="r12c")  # [m(row1/2), h]
        nc.vector.scalar_tensor_tensor(out=r12c[:, 0, :], in0=sqk4[:, 2:4],
                                       scalar=sgb[:, 5:6], op0=AL.mult,
                                       op1=AL.add, in1=th4[:, 2:4, 1])
        nc.vector.scalar_tensor_tensor(out=r12c[:, 0, :], in0=bqk4[:, 2:4],
                                       scalar=sgb[:, 1:2], op0=AL.mult,
                                       op1=AL.add, in1=r12c[:, 0, :])
        nc.vector.scalar_tensor_tensor(out=r12c[:, 1, :], in0=sqk4[:, 2:4],
                                       scalar=sgb[:, 2:3], op0=AL.mult,
                                       op1=AL.add, in1=th4[:, 2:4, 0])
        nc.vector.scalar_tensor_tensor(out=r12c[:, 1, :], in0=bqk4[:, 2:4],
                                       scalar=TCORE, op0=AL.mult,
                                       op1=AL.add, in1=r12c[:, 1, :])
        # per head: pack cols (alpha, beta | r1, r2), transpose to rows,
        # then two K=2 rank-1 matmuls finish the G accumulation group
        abr_sb = post.tile([128, 2, 4], BF, tag="abr")
        for h in range(2):
            nc.vector.tensor_scalar(abr_sb[:, h, 0:1], th4[:, h, 2:3], 1.0,
                                    None, AL.mult)
            nc.vector.tensor_scalar(abr_sb[:, h, 1:2], th4[:, h, 0:1], 1.0,
                                    None, AL.mult)
        nc.vector.tensor_scalar(abr_sb[:, :, 2], r12c[:, 0, :], 1.0, None,
                                AL.mult)
        nc.vector.tensor_scalar(abr_sb[:, :, 3], r12c[:, 1, :], 1.0, None,
                                AL.mult)
        ab_row = post.tile([2, 2, 128], BF, tag="ab_row")
        r12_row = post.tile([2, 2, 128], BF, tag="r12_row")
        for h in range(2):
            nc.tensor.transpose(tr_ps[0:2, 0, :], abr_sb[:, h, 0:2],
                                eye_sb[:])
            nc.tensor.transpose(tr_ps[0:2, 1, :], abr_sb[:, h, 2:4],
                                eye_sb[:])
            nc.scalar.activation(out=ab_row[:, h, :], in_=tr_ps[0:2, 0, :],
                                 func=AF.Copy)
            nc.scalar.activation(out=r12_row[:, h, :], in_=tr_ps[0:2, 1, :],
                                 func=AF.Copy)
        for h in range(2):
            nc.tensor.matmul(out=g_ps[:, h, :], lhsT=ab_row[:, h, :],
                             rhs=skbk_rows[:, h, :], start=False, stop=False)
            nc.tensor.matmul(out=g_ps[:, h, :], lhsT=sqbq_rows[:, h, :],
                             rhs=r12_row[:, h, :], start=False, stop=True)

        # pack [G0 | G1 | nq2 | nk2]  (bf16 collective)
        pack = post.tile([128, 260], BF, tag="pack")
        nc.scalar.activation(out=pack[:, 0:128], in_=g_ps[:, 0, :],
                             func=AF.Copy)
        nc.scalar.activation(out=pack[:, 128:256], in_=g_ps[:, 1, :],
                             func=AF.Copy)
        nc.vector.tensor_scalar(pack[:, 256:260], nqk2[:], 1.0, None, AL.mult)

    cc_in = dram.tile([128, 260], BF)
    cc_out = dram.tile([128, 260], BF)
    nc.sync.dma_start(out=cc_in[:, :], in_=pack[:])
    nc.gpsimd.collective_compute(
        "AllReduce", AL.add,
        replica_groups=[[0, 1], [2, 3], [4, 5], [6, 7]],
        ins=[cc_in.opt()], outs=[cc_out.opt()])

    red = post.tile([128, 260], BF, tag="red")
    nc.sync.dma_start(out=red[:], in_=cc_out[:, :])

    # ================= phase C: softmax + Pp/f assembly ================
    with tc.tile_pool(name="pc", bufs=1, space="PSUM") as pc2:
        e_ps = pc2.tile([128, 2, 2, 128], F32, tag="e_ps")
        # --- PE warmup chain to keep HAM hot across the collective gap ---
        wu_sb = post.tile([128, 128], BF, tag="wu_sb")
        nc.vector.tensor_scalar(wu_sb[:], eye_sb[:], 1.0, None, AL.mult)

        def warmup(n):
            for k in range(n):
                nc.tensor.matmul(out=e_ps[:, 0, 0, :], lhsT=wu_sb[:],
                                 rhs=eye_sb[:], start=True, stop=True)
                nc.scalar.activation(out=wu_sb[:], in_=e_ps[:, 0, 0, :],
                                     func=AF.Copy)

        warmup(18)

        trx_ps = pc2.tile([128, 6, 128], BF, tag="trx")
        tr2_ps = trx_ps[:, 0:2, :]
        invq = small.tile([128, 2], F32, tag="invq")
        invk = small.tile([128, 2], F32, tag="invk")
        for dst, src_off, mul_temp in ((invq, 256, True), (invk, 258, False)):
            sq_ = small.tile([128, 2], F32, tag="invn_sq")
            nc.scalar.activation(out=sq_[:], in_=red[:, src_off:src_off + 2],
                                 func=AF.Sqrt, bias=zb[:, :], scale=1.0)
            nc.vector.tensor_scalar_max(sq_[:], sq_[:], EPS_NORM)
            nc.vector.reciprocal(out=dst[:], in_=sq_[:])
            if mul_temp:
                nc.vector.tensor_tensor(out=dst[:], in0=dst[:],
                                        in1=temp_b[:, :], op=AL.mult)
        invk_bf = small.tile([128, 2], BF, tag="invk_bf")
        nc.vector.tensor_scalar(invk_bf[:], invk[:], 1.0, None, AL.mult)
        nc.tensor.transpose(tr2_ps[0:2, 0, :], invk_bf[:], eye_sb[:])
        ik2_sb = post.tile([2, 128], BF, tag="ik2")
        nc.scalar.activation(out=ik2_sb[:], in_=tr2_ps[0:2, 0, :],
                             func=AF.Copy)
        iktf_ps = pc2.tile([128, 2, 132], F32, tag="iktf_ps")
        ikb_ps = iktf_ps[:, :, 0:128]
        for h in range(2):
            nc.tensor.matmul(out=ikb_ps[:, h, :],
                             lhsT=sel4[0:2, h * 128:(h + 1) * 128],
                             rhs=ik2_sb[:], start=True, stop=True)

        a_sb = post.tile([128, 2, 128], BF, tag="a_sb")
        esc = post.tile([128, 2, 128], F32, tag="esc")
        for h in range(2):
            lh_t = post.tile([128, 128], F32, tag="lh_t")
            nc.vector.tensor_scalar(lh_t[:], red[:, h * 128:(h + 1) * 128],
                                    invq[:, h:h + 1], None, AL.mult)
            nc.vector.tensor_tensor(out=lh_t[:], in0=lh_t[:],
                                    in1=ikb_ps[:, h, :], op=AL.mult)
            rmax = small.tile([128, 1], F32, tag="rmax")
            nc.vector.tensor_reduce(out=rmax[:], in_=lh_t[:], op=AL.max,
                                    axis=AX.X)
            nc.vector.tensor_scalar(rmax[:], rmax[:], -1.0, None, AL.mult)
            rsum = small.tile([128, 1], F32, tag="rsum")
            nc.scalar.activation(out=esc[:, h, :], in_=lh_t[:], func=AF.Exp,
                                 bias=rmax[:, :], scale=1.0,
                                 accum_out=rsum[:])
            nc.vector.reciprocal(out=rsum[:], in_=rsum[:])
            nc.vector.tensor_scalar(a_sb[:, h, :], esc[:, h, :],
                                    rsum[:, :], None, AL.mult)

        attnT = post.tile([128, 2, 128], BF, tag="attnT")
        for h in range(2):
            nc.tensor.transpose(tr2_ps[:, h, :], a_sb[:, h, :], eye_sb[:])
        for h in range(2):
            nc.scalar.activation(out=attnT[:, h, :], in_=tr2_ps[:, h, :],
                                 func=AF.Copy)

        for h in range(2):
            for ph in range(2):
                nc.tensor.matmul(out=e_ps[:, ph, h, :],
                                 lhsT=woT[:, h, ph * 128:(ph + 1) * 128],
                                 rhs=a_sb[:, h, :], start=True, stop=True)
        e_sb = post.tile([128, 2, 2, 128], BF, tag="e_sb")
        for ph in range(2):
            nc.vector.tensor_scalar(e_sb[:, ph, 0, :], e_ps[:, ph, 0, :],
                                    1.0, None, AL.mult)
            nc.scalar.activation(out=e_sb[:, ph, 1, :], in_=e_ps[:, ph, 1, :],
                                 func=AF.Copy)
        et_ps = trx_ps[:, 2:6, :]
        eT_sb = post.tile([128, 2, 256], BF, tag="eT")
        for h in range(2):
            for ph in range(2):
                nc.tensor.transpose(et_ps[:, h * 2 + ph, :],
                                    e_sb[:, ph, h, :], eye_sb[:])
        for h in range(2):
            nc.vector.tensor_scalar(eT_sb[:, h, 0:128],
                                    et_ps[:, h * 2 + 0, :],
                                    1.0, None, AL.mult)
            nc.scalar.activation(out=eT_sb[:, h, 128:256],
                                 in_=et_ps[:, h * 2 + 1, :], func=AF.Copy)
        ppt_ps = pc2.tile([128, 2, 256], F32, tag="ppt")
        for mh in range(2):
            for h in range(2):
                nc.tensor.matmul(out=ppt_ps[:, mh, :],
                                 lhsT=wvr[:, h, mh * 128:(mh + 1) * 128],
                                 rhs=eT_sb[:, h, :],
                                 start=(h == 0), stop=(h == 1))
        pptT = post.tile([128, 2, 256], BF, tag="pptT")
        nc.vector.tensor_scalar(pptT[:, 0, :], ppt_ps[:, 0, :], 1.0, None,
                                AL.mult)
        nc.scalar.activation(out=pptT[:, 1, :], in_=ppt_ps[:, 1, :],
                             func=AF.Copy)

        t_ps = iktf_ps[:, :, 128:130]
        f12_ps = iktf_ps[:, :, 130:132]
        for h in range(2):
            nc.tensor.matmul(out=t_ps[:, h, :], lhsT=attnT[:, h, :],
                             rhs=svbv[:, h, :], start=True, stop=True)
        t_sb = post.tile([128, 2, 2], BF, tag="t_sb")
        nc.vector.tensor_scalar(t_sb[:], t_ps[:, :, :], 1.0, None, AL.mult)
        for ph in range(2):
            for h in range(2):
                nc.tensor.matmul(out=f12_ps[:, ph, :],
                                 lhsT=woT[:, h, ph * 128:(ph + 1) * 128],
                                 rhs=t_sb[:, h, :],
                                 start=(h == 0), stop=(h == 1))
        f12_sb = post.tile([128, 2, 2], BF, tag="f12sb")
        nc.vector.tensor_scalar(f12_sb[:, :, 0], f12_ps[:, :, 0], 1.0, None,
                                AL.mult)
        nc.vector.tensor_tensor(out=f12_sb[:, :, 1], in0=f12_ps[:, :, 1],
                                in1=bo_col[:, :], op=AL.add)
        # f rows: transpose [128,(ph,m)] -> [4,128] (k=ph*2+m), then
        # indicator-MM broadcasts; f2 becomes a K=4 block rhs for phase D
        nc.tensor.transpose(tr2_ps[0:4, 0, :], f12_sb[:, :, :], eye_sb[:])
        f4_sb = post.tile([4, 128], BF, tag="f4_sb")
        nc.scalar.activation(out=f4_sb[:], in_=tr2_ps[0:4, 0, :], func=AF.Copy)
        for ph in range(2):
            nc.tensor.matmul(out=ikb_ps[:, ph, :],
                             lhsT=sel4[:, (ph * 2) * 128:(ph * 2 + 1) * 128],
                             rhs=f4_sb[:], start=True, stop=True)
        f1b = post.tile([128, 256], F32, tag="f1b")
        nc.vector.tensor_scalar(f1b[:], ikb_ps[:, :, :], 1.0, None, AL.mult)
        f24_sb = post.tile([128, 256], BF, tag="f24")
        nc.vector.memset(f24_sb[:], 0.0)
        nc.vector.tensor_tensor(out=f24_sb[0:4, 0:128], in0=f4_sb[:],
                                in1=fm0[:], op=AL.mult)
        nc.vector.tensor_tensor(out=f24_sb[0:4, 128:256], in0=f4_sb[:],
                                in1=fm1[:], op=AL.mult)

        # ============= phase D: output pass (same psum pool) =============
        opsum0 = pc2.tile([128, 256], F32, tag="opsum0")
        opsum1 = pc2.tile([128, 256], F32, tag="opsum1")
        opsum2 = pc2.tile([128, 256], F32, tag="opsum2")
        opsum3 = pc2.tile([128, 256], F32, tag="opsum3")
        op_t = [opsum0, opsum1, opsum2, opsum3]
        for g in range(NCH):
            j0 = g * CHT
            out_sb = outp.tile([128, CHT, 256], BF, tag="out_sb")
            for jj in range(CHT):
                j = j0 + jj
                opsum = op_t[j % 4]
                nc.tensor.matmul(out=opsum[:], lhsT=rT_all[:, j, 0, :],
                                 rhs=pptT[:, 0, :], start=True, stop=False)
                nc.tensor.matmul(out=opsum[:], lhsT=rT_all[:, j, 1, :],
                                 rhs=pptT[:, 1, :], start=False, stop=False)
                nc.tensor.matmul(out=opsum[:], lhsT=m24[:, :],
                                 rhs=f24_sb[:, :], start=False, stop=True)
                nc.vector.scalar_tensor_tensor(
                    out=out_sb[:, jj, :], in0=f1b[:],
                    scalar=c_col[:, j:j + 1], op0=AL.mult, op1=AL.add,
                    in1=opsum[:])
            nc.sync.dma_start(out=outv[:, j0:j0 + CHT, :], in_=out_sb[:])


def _bcfree(tile_, col, n):
    """AP reading tile_[:, col] broadcast n times along free (0-stride)."""
    ap = tile_[:, col:col + 1]
    return bass.AP(tensor=ap.tensor, offset=ap.offset,
                   ap=[list(ap.ap[0])] + [[0, n]])


# ======================= host side =======================

def _sel4():
    s = np.zeros((4, 512), np.float32)
    for k in range(4):
        s[k, k * 128:(k + 1) * 128] = 1.0
    return s.astype(BF16)


def _m24():
    m = np.zeros((128, 128), np.float32)
    m[1, :] = 1.0
    m[3, :] = 1.0
    return m.astype(BF16)


def _prep_shared(inputs):
    f32 = np.float32
    Wq = np.asarray(inputs["Wq"], f32)
    bq = np.asarray(inputs["bq"], f32)
    Wkv = np.asarray(inputs["Wkv"], f32)
    bkv = np.asarray(inputs["bkv"], f32)
    Wo = np.asarray(inputs["Wo"], f32)
    bo = np.asarray(inputs["bo"], f32)
    lnS_w = np.asarray(inputs["lnS_w"], f32)
    lnS_b = np.asarray(inputs["lnS_b"], f32)
    lnR_w = np.asarray(inputs["lnR_w"], f32)
    lnR_b = np.asarray(inputs["lnR_b"], f32)
    temp = np.asarray(inputs["temperature"], f32).reshape(H)

    Wk, Wv = Wkv[:D], Wkv[D:]
    WqS = Wq * lnS_w[None, :]
    WkR = Wk * lnR_w[None, :]
    WvR = Wv * lnR_w[None, :]
    sq = WqS.sum(1)
    sk = WkR.sum(1)
    sv = WvR.sum(1)
    bq2 = Wq @ lnS_b + bq
    bk2 = Wk @ lnR_b + bkv[:D]
    bv2 = Wv @ lnR_b + bkv[D:]

    def halved(M):  # [256, X] -> [128, 2, X] rows split into halves
        return np.ascontiguousarray(
            M.reshape(2, 128, M.shape[1]).transpose(1, 0, 2)).astype(BF16)

    def colh(v):
        return np.ascontiguousarray(v.reshape(2, 128).T, f32)

    svbv = np.stack([sv, bv2], 1)  # [256, 2]
    sqk4 = np.concatenate([colh(sq), colh(sk)], 1)
    bqk4 = np.concatenate([colh(bq2), colh(bk2)], 1)
    return {
        "wqsT": halved(np.ascontiguousarray(WqS.T)),
        "wkrT": halved(np.ascontiguousarray(WkR.T)),
        "wqs": halved(WqS),
        "wkr": halved(WkR),
        "wvr": halved(WvR),
        "woT": halved(np.ascontiguousarray(Wo.T)),
        "svbv": halved(svbv),
        "sqk4": np.ascontiguousarray(sqk4),
        "bqk4": np.ascontiguousarray(bqk4),
        "bo_col": colh(bo),
        "skbk": np.stack([sk.reshape(2, 128),
                          bk2.reshape(2, 128)], 0).astype(BF16),
        "sqbq": np.stack([sq.reshape(2, 128),
                          bq2.reshape(2, 128)], 0).astype(BF16),
        "eye": np.eye(128).astype(BF16),
        "sel4": _sel4(),
        "m24": _m24(),
        "temp": temp.reshape(1, H).astype(f32),
    }


def _get_nc():
    if "nc" not in _nc_cache:
        _nc_cache["nc"] = _build_nc()
    return _nc_cache["nc"]


def run(inputs, trace=False):
    nc = _get_nc()
    shared = _prep_shared(inputs)
    iR = np.asarray(inputs["input_R"], np.float32)
    iS = np.asarray(inputs["input_S"], np.float32)
    in_maps = []
    for ci in range(N_CORES):
        b, half = ci // 2, ci % 2
        m = dict(shared)
        m["x_r"] = np.ascontiguousarray(iR[b, half * T:(half + 1) * T])
        m["x_s"] = np.ascontiguousarray(iS[b, half * T:(half + 1) * T])
        in_maps.append(m)
    res = run_bass_kernel_spmd(nc, in_maps, list(range(N_CORES)), trace=trace)
    out = np.zeros((B, N, D), np.float32)
    for ci in range(N_CORES):
        b, half = ci // 2, ci % 2
        out[b, half * T:(half + 1) * T] = np.asarray(
            res.results[ci]["out"]).astype(np.float32)
    return out, res


def kernel(**inputs):
    out, _ = run(inputs, trace=False)
    return out
